# revision 1
# baseline (speedup 1.0000x reference)
"""Trainium2 Bass kernel for nn_AttnBlock (B=16, C=512, H=W=32).

Strategy
--------
Data-parallel over batch: 16 batch elements / 8 NeuronCores = 2 per core.
Per batch element (C=512 channels, N=1024 pixels), all on one core:

  1. GroupNorm(32 groups) in [c, n] layout: per-partition bn_stats, then
     group aggregation / broadcast-back through tiny PE matmuls with 0/1
     indicator matrices (exact fp32).  Apply is one fused ACT pass
     hn = x*A + B with per-partition A, B.
  2. q = Wq hn, k = Wk hn in [c, n] layout; vT = (Wv hn)^T in [n, c]
     layout — all fp32r matmuls (full PE rate, ~11-bit mantissa products).
  3. Transposed-scores attention, avoiding any 1024x1024 transpose:
     eT[j,i] = exp(k^T q / sqrt(C)) computed directly in [j, i] layout
     (softmax max-subtraction is skipped: with these normalized inputs
     scores are O(6), exp is safe in fp32).  Row sums r[i] = sum_j eT
     via a ones-vector matmul; attention output
     av[c,i] = (sum_j vT[j,c] eT[j,i] + bv[c] r[i]) / r[i]
     with the 1/r normalization fused into the PSUM->SBUF eviction.
  4. proj: y = Wo av + bo + x, residual add fused into eviction, bo
     folded into the GEMM as a rank-1 (K=1) matmul.

Matmul loops are ordered so consecutive matmuls share the stationary
operand (both 512-wide query chunks per weight load) to cut LDWEIGHTS
pressure.  The kernel graph is built once per process and reused.
"""
import contextlib
import os
import sys

for _p in ("/opt/trn_rl_repo",):
    if _p not in sys.path and os.path.isdir(_p):
        sys.path.append(_p)

import numpy as np

import concourse.bass as bass
import concourse.tile as tile
from concourse import mybir
from concourse.bass_utils import run_bass_kernel_spmd
from concourse.vector_clock import ScopedClock

F32 = mybir.dt.float32
F32R = mybir.dt.float32r
AF = mybir.ActivationFunctionType

NCORES = 8
B, C, N = 16, 512, 1024
H = W = 32
NB = B // NCORES          # batch elements per core
CT = C // 128             # channel tiles of 128
NT = N // 128             # pixel tiles of 128
IC = N // 512             # query chunks of 512
G, GS = 32, 16            # groups, channels per group
GPT = 128 // GS           # groups per 128-channel tile
EPS = 1e-6


class _TC(tile.TileContext):
    """TileContext with multi-wait instructions split for this walrus.

    The pinned walrus accepts at most one semaphore wait per instruction
    (two for EventSemaphore).  Tile's scheduler can attach several; the
    extras are moved onto no-op carriers committed immediately before on
    the same engine, which is semantically identical (engine streams are
    sequential).
    """

    def _commit_instruction(self, inst, lazy_reg_writes: bool = True):
        si = inst.sync_info
        cap = 2 if isinstance(inst, mybir.InstEventSemaphore) else 1
        if si is not None and si.on_wait and len(si.on_wait) > cap and \
                inst.engine != mybir.EngineType.Unassigned:
            waits = list(si.on_wait)
            inst.sync_info = mybir.SyncInfo(
                on_wait=waits[:cap], on_update=list(si.on_update or [])
            )
            for w in waits[cap:]:
                nop = mybir.InstNoOp(
                    name=self.nc.get_next_instruction_name(),
                    ins=[],
                    outs=[],
                    engine=inst.engine,
                    sync_info=mybir.SyncInfo(on_wait=[w], on_update=[]),
                    bass_nofuse=True,
                )
                super()._commit_instruction(nop, lazy_reg_writes=False)
        super()._commit_instruction(inst, lazy_reg_writes)

    def _drain_and_barrier(self, tick_clock, wait_clock):
        # Collect the final-tick waits on a probe drain, then distribute
        # them across all engines (one wait per carrier instruction).
        # Each engine then signals a star-barrier semaphore; gpsimd
        # collects all signals and clears the semaphores.  This replaces
        # Tile's two EVSEM-butterfly all-engine barriers (~10us).
        nc = self.nc
        drain_inst = nc.sync.drain()
        wait_clock.add_sem_waits(
            drain_inst.ins, ScopedClock({None: tick_clock.global_clock})
        )
        si = drain_inst.ins.sync_info
        waits = list(si.on_wait) if si and si.on_wait else []
        drain_inst.ins.sync_info = mybir.SyncInfo(
            on_wait=waits[:1], on_update=[]
        )
        engines = list(nc.engines.values())
        for i, w in enumerate(waits[1:]):
            eng = engines[i % len(engines)]
            nop = eng.nop(nofuse=True)
            nop.ins.sync_info = mybir.SyncInfo(on_wait=[w], on_update=[])
        star = nc.alloc_semaphore("tile_star_barrier")
        nsig = 0
        for eng in engines:
            if eng is not nc.gpsimd:
                eng.sem_inc(star, 1)
                nsig += 1
        nc.gpsimd.wait_ge(star, nsig)
        assert self.sems is not None
        popped = nc._tile_sem_poison_stack.pop()
        assert popped is self._sem_poison
        nc.clear_and_free_semaphores(
            list(self.sems.allocated().values()) + [star])


def build_nc(use_bv: bool, use_bo: bool):
    nc = bass.Bass()

    # Per-core DRAM I/O.  Activations ship pre-arranged [p, ct, n].
    x_d = nc.declare_dram_parameter("x", [NB, 128, CT, N], F32, isOutput=False)
    y_d = nc.declare_dram_parameter("y", [NB, 128, CT, N], F32, isOutput=True)
    w_d = {
        name: nc.declare_dram_parameter(name, [128, CT, 512], F32R, isOutput=False)
        for name in ("wqT", "wkT", "wvT", "woT")
    }
    # pk1 packs [S | nsc | nbi | bqt | bkt] columns; pk2 packs the f32r row
    # constants [ones1(128) | ones512(512) | bor(512) | bvr(512)].
    pk1_d = nc.declare_dram_parameter("pk1", [128, GPT + 4 * CT], F32,
                                      isOutput=False)
    pk2_d = nc.declare_dram_parameter("pk2", [1, 128 + 3 * 512], F32R,
                                      isOutput=False)
    ST_d = nc.declare_dram_parameter("ST", [GPT, 128], F32, isOutput=False)
    ones_d = nc.declare_dram_parameter("ones", [128, 1], F32R, isOutput=False)

    scale = float(C) ** -0.5

    with _TC(nc) as tc:
        with (
            tc.tile_pool(name="consts", bufs=1) as consts,
            tc.tile_pool(name="big", bufs=1) as big,
            tc.tile_pool(name="small", bufs=2) as small,
            tc.tile_pool(name="psum", bufs=1, space="PSUM") as psum,
        ):
            # Packed constants on the scalar queue (4 tiny transfers); x of
            # batch 0 alone on the sync queue so groupnorm starts ASAP;
            # weights stream on the gpsimd queue.  DMA triggers cost ~600ns
            # each on the issuing engine.
            pk1_sb = consts.tile([128, GPT + 4 * CT], F32, tag="pk1")
            nc.scalar.dma_start(out=pk1_sb, in_=pk1_d[:, :])
            ST_sb = consts.tile([GPT, 128], F32, tag="ST")
            nc.scalar.dma_start(out=ST_sb, in_=ST_d[:, :])
            S_sb = pk1_sb[:, 0:GPT]
            nsc_sb = pk1_sb[:, GPT:GPT + CT]
            nbi_sb = pk1_sb[:, GPT + CT:GPT + 2 * CT]
            bqt_sb = pk1_sb[:, GPT + 2 * CT:GPT + 3 * CT]
            bkt_sb = pk1_sb[:, GPT + 3 * CT:GPT + 4 * CT]
            # Batch-0 x gets all three DMA queues first (it gates the whole
            # pipeline); weights follow, split across queues in the order
            # the GEMMs need them; batch-1 x drains last on gpsimd.
            x_sb0 = big.tile([128, CT, N], F32, tag="x", bufs=2, name="x_sb0")
            x_engs = [nc.sync, nc.sync, nc.sync, nc.scalar,
                      nc.scalar, nc.scalar, nc.gpsimd, nc.gpsimd]
            for ct in range(CT):
                for h in range(2):
                    x_engs[2 * ct + h].dma_start(
                        out=x_sb0[:, ct, h * 512:(h + 1) * 512],
                        in_=x_d[0, :, ct, h * 512:(h + 1) * 512])
            ones_sb = consts.tile([128, 1], F32R, tag="ones")
            nc.scalar.dma_start(out=ones_sb, in_=ones_d[:, :])
            pk2_sb = consts.tile([1, 128 + 3 * 512], F32R, tag="pk2")
            nc.scalar.dma_start(out=pk2_sb, in_=pk2_d[:, :])
            ones1_sb = pk2_sb[:, 0:128]
            ones512_sb = pk2_sb[:, 128:640]
            bor_sb = pk2_sb[:, 640:1152]
            bvr_sb = pk2_sb[:, 1152:1664]
            w_sb = {}
            w_engs = {"wqT": nc.gpsimd, "wkT": nc.sync,
                      "wvT": nc.gpsimd, "woT": nc.scalar}
            for name in ("wqT", "wkT", "wvT", "woT"):
                w_sb[name] = consts.tile([128, CT, 512], F32R, tag=name,
                                         name=f"w_{name}")
                w_engs[name].dma_start(out=w_sb[name], in_=w_d[name][:, :, :])
            eps_sb = consts.tile([GPT, 1], F32, tag="eps")
            nc.vector.memset(eps_sb, EPS)
            # Warm the ACT Sqrt table while DMAs stream, so the batch-0
            # groupnorm join does not pay the table load.
            sqrt_warm = consts.tile([GPT, 1], F32, tag="sqrt_warm")
            nc.scalar.activation(out=sqrt_warm, in_=eps_sb, func=AF.Sqrt,
                                 bias=eps_sb, scale=1.0)

            for b in range(NB):
                # ---- load x ----
                if b == 0:
                    x_sb = x_sb0
                else:
                    # Later batches load on the scalar queue behind the small
                    # constants: naturally delayed past batch 0's critical
                    # x DMAs, still well ahead of this batch's groupnorm.
                    # (woT is queued after this and is needed even later.)
                    x_sb = big.tile([128, CT, N], F32, tag="x", bufs=2,
                                    name=f"x_sb{b}")
                    for ct in range(CT):
                        nc.scalar.dma_start(out=x_sb[:, ct], in_=x_d[b, :, ct])

                # ---- GroupNorm statistics, per channel tile ----
                # For batches > 0, hoist the groupnorm chain's scheduler
                # priority back to the previous batch's qkv phase: its x is
                # resident by then, and finishing the join early lets this
                # batch's qkv matmuls fill the previous batch's attention
                # tail, keeping the PE HAM-warm.
                # Private tiles per ct: a shared tile would add false
                # whole-tile dependencies that stall the first stats matmul.
                gstats = small.tile([GPT, CT, 2], F32, tag="gstats")
                # channel tiles processed in DMA-arrival order (queue split
                # above): DVE is in-order, so matching arrival avoids
                # stalls.  tile_wait_until feeds the scheduler the real DMA
                # arrival times, which its cost model cannot see; without it
                # the DVE stream interleaves chains and the first stats
                # matmul inherits a late semaphore tick.
                # Floor ONLY the bn_stats ops at their true DMA-arrival
                # times: the scheduler's cost model thinks DMA is instant,
                # so without this it orders late-arriving tiles' bn_stats
                # ahead of earlier tiles' tiny follow-up ops in the in-order
                # DVE stream, stalling the first stats matmuls ~8us.
                arrive_ms = {(0, 0): 0.010, (0, 1): 0.0105,
                             (3, 0): 0.011, (3, 1): 0.0115,
                             (1, 0): 0.012, (1, 1): 0.0145,
                             (2, 0): 0.015, (2, 1): 0.017}
                for ct in ((0, 3, 1, 2) if b == 0 else range(CT)):
                    with tc.tile_wait_until(0, enable=False):
                        stats = small.tile([128, 2, 6], F32, tag=f"bnst{ct}",
                                           name=f"bnst_{b}_{ct}")
                        ts = small.tile([128, 2], F32, tag=f"ts{ct}",
                                        name=f"ts_{b}_{ct}")
                        mv = small.tile([128, 2], F32, tag=f"mv{ct}",
                                        name=f"mv_{b}_{ct}")
                        for h in range(2):
                            with tc.tile_wait_until(arrive_ms[(ct, h)],
                                                    enable=False):
                                nc.vector.bn_stats(
                                    out=stats[:, h],
                                    in_=x_sb[:, ct, h * 512:(h + 1) * 512],
                                )
                        nc.vector.bn_aggr(out=mv, in_=stats)
                        nc.vector.tensor_copy(ts[:, 0:1], mv[:, 0:1])
                        nc.vector.tensor_mul(ts[:, 1:2], mv[:, 0:1], mv[:, 0:1])
                        nc.vector.tensor_add(ts[:, 1:2], ts[:, 1:2], mv[:, 1:2])
                        ps = psum.tile([GPT, 2], F32, tag="mm", bufs=6,
                                       name=f"stat_ps_{b}_{ct}")
                        nc.tensor.matmul(ps, lhsT=S_sb, rhs=ts,
                                         start=True, stop=True)
                        nc.vector.tensor_copy(gstats[:, ct], ps)
                # join: group mean / rstd for all tiles at once
                gm = small.tile([GPT, CT, 2], F32, tag="gm")
                nc.vector.tensor_scalar_mul(gm[:, :, 0], gstats[:, :, 0], 1.0 / GS)
                nc.vector.tensor_scalar_mul(gm[:, :, 1], gstats[:, :, 1], 1.0 / GS)
                tmp8 = small.tile([GPT, CT], F32, tag="tmp8")
                nc.vector.tensor_mul(tmp8, gm[:, :, 0], gm[:, :, 0])
                nc.vector.tensor_sub(gm[:, :, 1], gm[:, :, 1], tmp8)
                nc.scalar.activation(out=gm[:, :, 1], in_=gm[:, :, 1],
                                     func=AF.Sqrt, bias=eps_sb, scale=1.0)
                nc.vector.reciprocal(gm[:, :, 1], gm[:, :, 1])
                # broadcast (mean_g, rstd_g) back to the 128 channel rows
                AB = small.tile([128, CT, 2], F32, tag="AB")
                for ct in range(CT):
                    ps = psum.tile([128, 2], F32, tag="mm", bufs=6,
                                   name=f"ab_ps_{b}_{ct}")
                    nc.tensor.matmul(ps, lhsT=ST_sb, rhs=gm[:, ct],
                                     start=True, stop=True)
                    nc.vector.tensor_copy(AB[:, ct], ps)
                A_sb = small.tile([128, CT], F32, tag="A")
                B_sb = small.tile([128, CT], F32, tag="B")
                nc.vector.tensor_mul(A_sb, AB[:, :, 1], nsc_sb)
                nc.vector.tensor_mul(B_sb, AB[:, :, 0], A_sb)
                nc.vector.tensor_sub(B_sb, nbi_sb, B_sb)
                # hn = x*A + B, rounded to f32r for the GEMMs
                # (shares slots with eT: hn is dead once q/k/vT are built)
                hn_sb = big.tile([128, CT, N], F32R, tag="hn_eT", bufs=2,
                                 name=f"hn_sb{b}")
                for ct in range(CT):
                    # split the apply across DVE and ACT so the four tiles
                    # finish in half the time
                    if ct % 2 == 0:
                        nc.vector.tensor_scalar(
                            out=hn_sb[:, ct], in0=x_sb[:, ct],
                            scalar1=A_sb[:, ct:ct + 1],
                            scalar2=B_sb[:, ct:ct + 1],
                            op0=mybir.AluOpType.mult, op1=mybir.AluOpType.add,
                        )
                    else:
                        nc.scalar.activation(
                            out=hn_sb[:, ct], in_=x_sb[:, ct],
                            func=AF.Identity, scale=A_sb[:, ct:ct + 1],
                            bias=B_sb[:, ct:ct + 1],
                        )

                # ---- q, k in [c, n] layout (paired query chunks) ----
                q_sb = big.tile([128, CT, N], F32R, tag="q")
                k_sb = big.tile([128, CT, N], F32R, tag="k")
                for wname, dst in (("wqT", q_sb), ("wkT", k_sb)):
                    for ot in range(CT):
                        pss = [psum.tile([128, 512], F32, tag="mm", bufs=6,
                                         name=f"{wname}_ps_{b}_{ot}_{ic}")
                               for ic in range(IC)]
                        for ct in range(CT):
                            for ic in range(IC):
                                nc.tensor.matmul(
                                    pss[ic],
                                    lhsT=w_sb[wname][:, ct, ot * 128:(ot + 1) * 128],
                                    rhs=hn_sb[:, ct, ic * 512:(ic + 1) * 512],
                                    start=(ct == 0), stop=(ct == CT - 1),
                                )
                        bias_sb = bqt_sb if wname == "wqT" else bkt_sb
                        for ic in range(IC):
                            nc.vector.tensor_scalar_add(
                                dst[:, ot, ic * 512:(ic + 1) * 512],
                                pss[ic], bias_sb[:, ot:ot + 1])

                # ---- vT in [n, c] layout ----
                vT_sb = big.tile([128, NT, 512], F32R, tag="vT")
                for nt in range(NT):
                    ps = psum.tile([128, 512], F32, tag="mm", bufs=6,
                                   name=f"v_ps_{b}_{nt}")
                    for ct in range(CT):
                        nc.tensor.matmul(
                            ps,
                            lhsT=hn_sb[:, ct, nt * 128:(nt + 1) * 128],
                            rhs=w_sb["wvT"][:, ct, :],
                            start=(ct == 0), stop=(ct == CT - 1),
                        )
                    nc.vector.tensor_copy(vT_sb[:, nt], ps)

                # ---- scores + exp for both query chunks ----
                eTs = [big.tile([128, NT, 512], F32R, tag="hn_eT", bufs=2,
                                name=f"eT_sb_{b}_{ic}") for ic in range(IC)]
                for jt in range(NT):
                    pss = [psum.tile([128, 512], F32, tag="mm", bufs=6,
                                     name=f"sc_ps_{b}_{jt}_{ic}")
                           for ic in range(IC)]
                    for ct in range(CT):
                        for ic in range(IC):
                            nc.tensor.matmul(
                                pss[ic],
                                lhsT=k_sb[:, ct, jt * 128:(jt + 1) * 128],
                                rhs=q_sb[:, ct, ic * 512:(ic + 1) * 512],
                                start=(ct == 0), stop=(ct == CT - 1),
                            )
                    for ic in range(IC):
                        nc.scalar.activation(
                            out=eTs[ic][:, jt], in_=pss[ic], func=AF.Exp,
                            scale=scale, bias=0.0,
                        )
                # r[i] = sum_j eT[j, i]; 1/r = exp(-ln(r)) on ACT: r is
                # strictly positive, and the DVE reciprocal's ~6 cycles per
                # element on a 512-long row would sit on the critical path.
                rs_pss = [psum.tile([1, 512], F32, tag="small", bufs=2,
                                    name=f"rs_ps_{b}_{ic}") for ic in range(IC)]
                for jt in range(NT):
                    for ic in range(IC):
                        nc.tensor.matmul(rs_pss[ic], lhsT=ones_sb,
                                         rhs=eTs[ic][:, jt],
                                         start=(jt == 0), stop=(jt == NT - 1))
                rsums, rinvs = [], []
                for ic in range(IC):
                    lr_sb = small.tile([1, 512], F32, tag="lnr", bufs=2,
                                       name=f"lnr_{b}_{ic}")
                    nc.scalar.activation(out=lr_sb, in_=rs_pss[ic], func=AF.Ln)
                    rinv_sb = small.tile([1, 512], F32R, tag="rinv", bufs=2,
                                         name=f"rinv_{b}_{ic}")
                    nc.scalar.activation(out=rinv_sb, in_=lr_sb, func=AF.Exp,
                                         scale=-1.0)
                    rinvs.append(rinv_sb)
                    if use_bv:
                        rsum_sb = small.tile([1, 512], F32R, tag="rsum",
                                             bufs=2, name=f"rsum_{b}_{ic}")
                        nc.vector.tensor_copy(rsum_sb, rs_pss[ic])
                        rsums.append(rsum_sb)

                # ---- av[c,i] = (sum_j vT[j,c] eT[j,i] [+ bv r]) / r ----
                avns = [big.tile([128, CT, 512], F32R, tag="avn", bufs=2,
                                 name=f"avn_{b}_{ic}") for ic in range(IC)]
                av_pss = []
                bc_pss = []
                for ct in range(CT):
                    pss = [psum.tile([128, 512], F32, tag="mm", bufs=6,
                                     name=f"av_ps_{b}_{ct}_{ic}")
                           for ic in range(IC)]
                    av_pss.append(pss)
                    for jt in range(NT):
                        for ic in range(IC):
                            nc.tensor.matmul(
                                pss[ic],
                                lhsT=vT_sb[:, jt, ct * 128:(ct + 1) * 128],
                                rhs=eTs[ic][:, jt],
                                start=(jt == 0),
                                stop=(jt == NT - 1 and not use_bv),
                            )
                    if use_bv:
                        for ic in range(IC):
                            nc.tensor.matmul(
                                pss[ic],
                                lhsT=bvr_sb[0:1, ct * 128:(ct + 1) * 128],
                                rhs=rsums[ic], start=False, stop=True,
                            )
                    if ct == 0:
                        # broadcast 1/r across partitions; placed after the
                        # first AV group so the PE does not idle on the DVE
                        # reciprocal above.
                        for ic in range(IC):
                            bc_ps = psum.tile([128, 512], F32, tag="mm",
                                              bufs=6, name=f"bc_ps_{b}_{ic}")
                            nc.tensor.matmul(bc_ps, lhsT=ones1_sb,
                                             rhs=rinvs[ic],
                                             start=True, stop=True)
                            bc_pss.append(bc_ps)
                rinvbs = []
                for ic in range(IC):
                    rinvb_sb = small.tile([128, 512], F32, tag="rinvb", bufs=2,
                                          name=f"rinvb_{b}_{ic}")
                    nc.vector.tensor_copy(rinvb_sb, bc_pss[ic])
                    rinvbs.append(rinvb_sb)
                for ct in range(CT):
                    for ic in range(IC):
                        nc.vector.tensor_mul(avns[ic][:, ct], av_pss[ct][ic],
                                             rinvbs[ic])

                # ---- y = Wo av + bo + x ----
                for ot in range(CT):
                    pss = [psum.tile([128, 512], F32, tag="mm", bufs=6,
                                     name=f"pr_ps_{b}_{ot}_{ic}")
                           for ic in range(IC)]
                    for ct in range(CT):
                        for ic in range(IC):
                            nc.tensor.matmul(
                                pss[ic],
                                lhsT=w_sb["woT"][:, ct, ot * 128:(ot + 1) * 128],
                                rhs=avns[ic][:, ct],
                                start=(ct == 0),
                                stop=(ct == CT - 1 and not use_bo),
                            )
                    if use_bo:
                        for ic in range(IC):
                            nc.tensor.matmul(
                                pss[ic], lhsT=bor_sb[0:1, ot * 128:(ot + 1) * 128],
                                rhs=ones512_sb, start=False, stop=True,
                            )
                    for ic in range(IC):
                        y_sb = big.tile([128, 512], F32, tag="y", bufs=3,
                                        name=f"y_{b}_{ot}_{ic}")
                        nc.vector.tensor_add(
                            y_sb, pss[ic], x_sb[:, ot, ic * 512:(ic + 1) * 512]
                        )
                        nc.sync.dma_start(
                            out=y_d[b, :, ot, ic * 512:(ic + 1) * 512], in_=y_sb
                        )
    return nc


_CACHE = {}


def _get_nc(use_bv=False, use_bo=False):
    key = (use_bv, use_bo)
    if key not in _CACHE:
        _CACHE[key] = build_nc(use_bv, use_bo)
    return _CACHE[key]


def prepare(x, norm_scale, norm_bias, wq, bq, wk, bk, wv, bv, wo, bo):
    """Host-side prep: returns (in_maps, use_bv, use_bo)."""
    x = np.ascontiguousarray(np.asarray(x, dtype=np.float32))
    f32 = lambda a: np.asarray(a, dtype=np.float32)
    norm_scale, norm_bias = f32(norm_scale), f32(norm_bias)
    wq, wk, wv, wo = f32(wq), f32(wk), f32(wv), f32(wo)
    bq, bk, bv, bo = f32(bq), f32(bk), f32(bv), f32(bo)

    # [C, C] w  ->  wT[c, o] arranged [p, ct, o]
    def arr_w(w):
        return np.ascontiguousarray(
            w.T.reshape(CT, 128, C).transpose(1, 0, 2))

    # [C] vec (channel-tile major) -> [p, ct]
    def arr_c(v):
        return np.ascontiguousarray(v.reshape(CT, 128).T)

    S = np.zeros((128, GPT), np.float32)
    S[np.arange(128), np.arange(128) // GS] = 1.0
    pk1 = np.concatenate(
        [S, arr_c(norm_scale), arr_c(norm_bias), arr_c(bq), arr_c(bk)], axis=1)
    pk2 = np.concatenate(
        [np.ones(128, np.float32), np.ones(512, np.float32),
         bo.reshape(C), bv.reshape(C)]).reshape(1, -1)
    common = {
        "wqT": arr_w(wq), "wkT": arr_w(wk), "wvT": arr_w(wv), "woT": arr_w(wo),
        "pk1": np.ascontiguousarray(pk1),
        "pk2": np.ascontiguousarray(pk2),
        "ST": np.ascontiguousarray(S.T),
        "ones": np.ones((128, 1), np.float32),
    }

    # x: (B, C, H, W) -> per core [NB, p, ct, n]
    xf = x.reshape(B, C, N).reshape(B, CT, 128, N).transpose(0, 2, 1, 3)
    in_maps = [
        {**common, "x": np.ascontiguousarray(xf[i * NB:(i + 1) * NB])}
        for i in range(NCORES)
    ]
    return in_maps, bool(np.any(bv != 0.0)), bool(np.any(bo != 0.0))


def assemble(results):
    y = np.empty((B, C, N), np.float32)
    for i in range(NCORES):
        yc = results[i]["y"]  # [NB, 128, CT, N]
        y[i * NB:(i + 1) * NB] = (
            yc.transpose(0, 2, 1, 3).reshape(NB, C, N))
    return y.reshape(B, C, H, W)


def kernel(x, norm_scale, norm_bias, wq, bq, wk, bk, wv, bv, wo, bo):
    in_maps, use_bv, use_bo = prepare(x, norm_scale, norm_bias, wq, bq,
                                      wk, bk, wv, bv, wo, bo)
    nc = _get_nc(use_bv=use_bv, use_bo=use_bo)
    res = run_bass_kernel_spmd(nc, in_maps, list(range(NCORES)))
    return assemble(res.results)



# revision 8
# speedup vs baseline: 1.1119x; 1.1119x over previous
"""Trainium2 Bass kernel for nn_AttnBlock (B=16, C=512, H=W=32).

Strategy
--------
Data-parallel over batch: 16 batch elements / 8 NeuronCores = 2 per core.
Per batch element (C=512 channels, N=1024 pixels), all on one core:

  1. GroupNorm(32 groups) in [c, n] layout, pipelined PER CHANNEL TILE:
     each 128-channel tile's stats (bn_stats -> group aggregation via a
     tiny 0/1-indicator PE matmul -> sqrt/reciprocal -> broadcast-back
     matmul) complete as soon as that tile's x DMA lands, and the
     hn = x*A + B apply for that tile follows immediately.  The q/k
     GEMMs accumulate channel tiles in DMA-arrival order, so the PE
     starts ~6us into the run instead of waiting for all of x.
  2. q = Wq hn, k = Wk hn, vT = (Wv hn)^T -- f32r matmuls at full PE
     rate.  All three evict to fp8e4m3 for the attention GEMMs.
  3. Attention entirely in fp8 DoubleRow matmuls (2 fp8 MACs per PE
     cell per cycle -- 2x the f32r rate): eT[j,i] = exp(kq/sqrt(C) - 2)
     computed directly in [j, i] layout (the -2 bias keeps exp <= 240,
     the TRN fp8e4 max; softmax normalization cancels it exactly).
     Row sums via a DoubleRow ones-vector matmul; 1/r via ACT ln/exp;
     av = (vT^T eT) * (1/r) evicted to fp8.
  4. proj: y = Wo av + x with Wo in fp8 DoubleRow and the residual x
     added INTO the proj PSUM by an identity-matrix f32r matmul, so the
     eviction is a pure copy (split DVE/ACT) and the vector engine
     stays off the critical tail.

Precision (sim, scale-relative absmax vs f32 reference): 9.9e-3, vs
the 2e-2 gate.  fp8 is applied only where the softmax structure damps
it (scores/eT/av/proj); groupnorm, q/k/v GEMM inputs stay f32r/bf16.

Matmul loops order consecutive matmuls to share the stationary operand.
The kernel graph is built once per process and reused.
"""
import contextlib
import os
import sys

for _p in ("/opt/trn_rl_repo",):
    if _p not in sys.path and os.path.isdir(_p):
        sys.path.append(_p)

import numpy as np
import ml_dtypes

import concourse.bass as bass
import concourse.tile as tile
from concourse import mybir
from concourse.bass_utils import run_bass_kernel_spmd
from concourse.vector_clock import ScopedClock

F32 = mybir.dt.float32
F32R = mybir.dt.float32r
BF16 = mybir.dt.bfloat16
F8 = mybir.dt.float8e4
AF = mybir.ActivationFunctionType
DR = mybir.MatmulPerfMode.DoubleRow

NCORES = 8
B, C, N = 16, 512, 1024
H = W = 32
NB = B // NCORES          # batch elements per core
CT = C // 128             # channel tiles of 128
NT = N // 128             # pixel tiles of 128
IC = N // 512             # query chunks of 512
CP = CT // 2              # channel-tile pairs (DoubleRow K=256)
JP = NT // 2              # pixel-tile pairs (DoubleRow K=256)
G, GS = 32, 16            # groups, channels per group
GPT = 128 // GS           # groups per 128-channel tile
EPS = 1e-6
EXP_BIAS = 2.0            # exp(s - 2): keeps eT <= ~125 < 240 (fp8e4 max)


class _TC(tile.TileContext):
    """TileContext with multi-wait instructions split for this walrus.

    The pinned walrus accepts at most one semaphore wait per instruction
    (two for EventSemaphore).  Tile's scheduler can attach several; the
    extras are moved onto no-op carriers committed immediately before on
    the same engine, which is semantically identical (engine streams are
    sequential).
    """

    def _commit_instruction(self, inst, lazy_reg_writes: bool = True):
        si = inst.sync_info
        cap = 2 if isinstance(inst, mybir.InstEventSemaphore) else 1
        if si is not None and si.on_wait and len(si.on_wait) > cap and \
                inst.engine != mybir.EngineType.Unassigned:
            waits = list(si.on_wait)
            inst.sync_info = mybir.SyncInfo(
                on_wait=waits[:cap], on_update=list(si.on_update or [])
            )
            for w in waits[cap:]:
                nop = mybir.InstNoOp(
                    name=self.nc.get_next_instruction_name(),
                    ins=[],
                    outs=[],
                    engine=inst.engine,
                    sync_info=mybir.SyncInfo(on_wait=[w], on_update=[]),
                    bass_nofuse=True,
                )
                super()._commit_instruction(nop, lazy_reg_writes=False)
        super()._commit_instruction(inst, lazy_reg_writes)

    def _drain_and_barrier(self, tick_clock, wait_clock):
        # Collect the final-tick waits on a probe drain, then distribute
        # them across all engines (one wait per carrier instruction).
        # Each engine then signals a star-barrier semaphore; gpsimd
        # collects all signals and clears the semaphores.  This replaces
        # Tile's two EVSEM-butterfly all-engine barriers (~10us).
        nc = self.nc
        drain_inst = nc.sync.drain()
        wait_clock.add_sem_waits(
            drain_inst.ins, ScopedClock({None: tick_clock.global_clock})
        )
        si = drain_inst.ins.sync_info
        waits = list(si.on_wait) if si and si.on_wait else []
        drain_inst.ins.sync_info = mybir.SyncInfo(
            on_wait=waits[:1], on_update=[]
        )
        engines = list(nc.engines.values())
        for i, w in enumerate(waits[1:]):
            eng = engines[i % len(engines)]
            nop = eng.nop(nofuse=True)
            nop.ins.sync_info = mybir.SyncInfo(on_wait=[w], on_update=[])
        star = nc.alloc_semaphore("tile_star_barrier")
        nsig = 0
        for eng in engines:
            if eng is not nc.gpsimd:
                eng.sem_inc(star, 1)
                nsig += 1
        nc.gpsimd.wait_ge(star, nsig)
        assert self.sems is not None
        popped = nc._tile_sem_poison_stack.pop()
        assert popped is self._sem_poison
        nc.clear_and_free_semaphores(
            list(self.sems.allocated().values()) + [star])


def build_nc(use_bq: bool, use_bk: bool, use_bv: bool, use_bo: bool):
    nc = bass.Bass()

    # Per-core DRAM I/O.  Activations ship pre-arranged [p, ct, n]; x is
    # declared f32r (same bits as f32) so the PE identity-matmul residual
    # add can read it at full rate.
    x_d = nc.declare_dram_parameter("x", [NB, 128, CT, N], F32R, isOutput=False)
    y_d = nc.declare_dram_parameter("y", [NB, 128, CT, N], F32, isOutput=True)
    wq_d = nc.declare_dram_parameter("wqT", [128, CT, 512], F32R, isOutput=False)
    wk_d = nc.declare_dram_parameter("wkT", [128, CT, 512], F32R, isOutput=False)
    wv_d = nc.declare_dram_parameter("wvT", [128, CT, 512], F32R, isOutput=False)
    wo_d = nc.declare_dram_parameter("woT8", [128, CT, 512], F8, isOutput=False)
    id_d = nc.declare_dram_parameter("idm", [128, 128], F32R, isOutput=False)
    # pk1 packs [S | nsc | nbi | bqt | bkt] f32 columns.
    pk1_d = nc.declare_dram_parameter("pk1", [128, GPT + 4 * CT], F32,
                                      isOutput=False)
    # pk2 packs the f32r row constants [ones1(128) | ones512(512) |
    # bor(512) | bvr(512)].
    pk2_d = nc.declare_dram_parameter("pk2", [1, 128 + 3 * 512], F32R,
                                      isOutput=False)
    ST_d = nc.declare_dram_parameter("ST", [GPT, 128], F32, isOutput=False)

    scale = float(C) ** -0.5

    with _TC(nc) as tc:
        with (
            tc.tile_pool(name="consts", bufs=1) as consts,
            tc.tile_pool(name="big", bufs=1) as big,
            tc.tile_pool(name="small", bufs=2) as small,
            tc.tile_pool(name="psum", bufs=1, space="PSUM") as psum,
        ):
            # --- constants: tiny transfers first on the scalar queue ---
            pk1_sb = consts.tile([128, GPT + 4 * CT], F32, tag="pk1")
            nc.scalar.dma_start(out=pk1_sb, in_=pk1_d[:, :])
            ST_sb = consts.tile([GPT, 128], F32, tag="ST")
            nc.scalar.dma_start(out=ST_sb, in_=ST_d[:, :])
            # ones for the DoubleRow row-sum; 16 columns because dual-fp8
            # LDWEIGHTS needs the pair-dim step to be a multiple of 16B.
            ones8_sb = consts.tile([128, 2, 16], F8, tag="ones8")
            nc.vector.memset(ones8_sb, 1.0)
            pk2_sb = consts.tile([1, 128 + 3 * 512], F32R, tag="pk2")
            nc.scalar.dma_start(out=pk2_sb, in_=pk2_d[:, :])
            S_sb = pk1_sb[:, 0:GPT]
            nsc_sb = pk1_sb[:, GPT:GPT + CT]
            nbi_sb = pk1_sb[:, GPT + CT:GPT + 2 * CT]
            bqt_sb = pk1_sb[:, GPT + 2 * CT:GPT + 3 * CT]
            bkt_sb = pk1_sb[:, GPT + 3 * CT:GPT + 4 * CT]
            ones1_sb = pk2_sb[:, 0:128]
            ones512_sb = pk2_sb[:, 128:640]
            bor_sb = pk2_sb[:, 640:1152]
            bvr_sb = pk2_sb[:, 1152:1664]

            # --- batch-0 x: 8 half-tile transfers spread over 6 queues
            # so the first channel tiles land ~3.5us in.  PE and DVE are
            # idle at t=0, so their ~600ns trigger cost is free.
            x_sbs = []
            x_sb0 = big.tile([128, CT, N], F32R, tag="x", bufs=2, name="x_sb0")
            x_sbs.append(x_sb0)
            xq = {(0, 0): nc.sync, (0, 1): nc.scalar,
                  (1, 0): nc.gpsimd, (1, 1): nc.sync,
                  (2, 0): nc.scalar, (2, 1): nc.gpsimd,
                  (3, 0): nc.sync, (3, 1): nc.scalar}
            for ct in range(CT):
                for h in range(2):
                    xq[(ct, h)].dma_start(
                        out=x_sb0[:, ct, h * 512:(h + 1) * 512],
                        in_=x_d[0, :, ct, h * 512:(h + 1) * 512])

            # --- weights: behind the x chunks on their queues, in
            # first-use order (wq/wk at PE start, wv ~4us later, wo last).
            id_sb = consts.tile([128, 128], F32R, tag="idm")
            nc.scalar.dma_start(out=id_sb, in_=id_d[:, :])
            wq_sb = consts.tile([128, CT, 512], F32R, tag="wq")
            nc.scalar.dma_start(out=wq_sb, in_=wq_d[:, :, :])
            wk_sb = consts.tile([128, CT, 512], F32R, tag="wk")
            nc.sync.dma_start(out=wk_sb, in_=wk_d[:, :, :])
            wv_sb = consts.tile([128, CT, 512], F32R, tag="wv")
            nc.gpsimd.dma_start(out=wv_sb, in_=wv_d[:, :, :])
            wo_sb = consts.tile([128, CT, 512], F8, tag="wo")
            nc.scalar.dma_start(out=wo_sb, in_=wo_d[:, :, :])

            eps_sb = consts.tile([GPT, 1], F32, tag="eps")
            nc.vector.memset(eps_sb, EPS)
            ebias_sb = consts.tile([128, 1], F32, tag="ebias")
            nc.vector.memset(ebias_sb, -EXP_BIAS)
            # Warm the ACT tables used later (first use of a function
            # pays the table load) while DMAs stream.
            for wf, wname in ((AF.Sqrt, "sqw"), (AF.Exp, "exw"),
                              (AF.Ln, "lnw"), (AF.Identity, "idw")):
                wt = consts.tile([GPT, 1], F32, tag=wname)
                nc.scalar.activation(out=wt, in_=eps_sb, func=wf,
                                     bias=0.0, scale=1.0)

            # Estimated DMA arrival (ms) of each batch-0 x half-tile; the
            # scheduler's cost model thinks DMA is instant, so floor the
            # dependent bn_stats at these times to keep the in-order DVE
            # stream from blocking early tiles behind late ones.
            arrive_ms = {(0, 0): 0.0030, (0, 1): 0.0030,
                         (1, 0): 0.0035, (1, 1): 0.0055,
                         (2, 0): 0.0055, (2, 1): 0.0060,
                         (3, 0): 0.0080, (3, 1): 0.0085}

            # Per-batch state carried across the phase interleave below.
            st = [dict() for _ in range(NB)]

            def phase_norm(b):
                """GroupNorm pipelined per channel tile + hn apply."""
                x_sb = x_sbs[b]
                A_sb = small.tile([128, CT], F32, tag="A", name=f"A{b}")
                B_sb = small.tile([128, CT], F32, tag="B", name=f"B{b}")
                hn_sb = big.tile([128, CT, N], F32R, tag="hn", bufs=2,
                                 name=f"hn{b}")
                for ct in range(CT):
                    stats = small.tile([128, 2, 6], F32, tag=f"bnst{ct}",
                                       name=f"bnst_{b}_{ct}")
                    ts = small.tile([128, 2], F32, tag=f"ts{ct}",
                                    name=f"ts_{b}_{ct}")
                    mv = small.tile([128, 2], F32, tag=f"mv{ct}",
                                    name=f"mv_{b}_{ct}")
                    for h in range(2):
                        with tc.tile_wait_until(
                                arrive_ms.get((ct, h), 0) if b == 0 else 0,
                                enable=False):
                            nc.vector.bn_stats(
                                out=stats[:, h],
                                in_=x_sb[:, ct, h * 512:(h + 1) * 512],
                            )
                    nc.vector.bn_aggr(out=mv, in_=stats)
                    nc.vector.tensor_copy(ts[:, 0:1], mv[:, 0:1])
                    nc.vector.tensor_mul(ts[:, 1:2], mv[:, 0:1], mv[:, 0:1])
                    nc.vector.tensor_add(ts[:, 1:2], ts[:, 1:2], mv[:, 1:2])
                    ps = psum.tile([GPT, 2], F32, tag="mm", bufs=6,
                                   name=f"stat_ps_{b}_{ct}")
                    nc.tensor.matmul(ps, lhsT=S_sb, rhs=ts,
                                     start=True, stop=True)
                    # group mean / rstd for this tile's 8 groups
                    gm = small.tile([GPT, 2], F32, tag=f"gm{ct}",
                                    name=f"gm_{b}_{ct}")
                    nc.vector.tensor_scalar_mul(gm, ps, 1.0 / GS)
                    tmp = small.tile([GPT, 1], F32, tag=f"tmp{ct}",
                                     name=f"tmp_{b}_{ct}")
                    nc.vector.tensor_mul(tmp, gm[:, 0:1], gm[:, 0:1])
                    nc.vector.tensor_sub(gm[:, 1:2], gm[:, 1:2], tmp)
                    nc.scalar.activation(out=gm[:, 1:2], in_=gm[:, 1:2],
                                         func=AF.Sqrt, bias=eps_sb, scale=1.0)
                    nc.vector.reciprocal(gm[:, 1:2], gm[:, 1:2])
                    ps2 = psum.tile([128, 2], F32, tag="mm", bufs=6,
                                    name=f"ab_ps_{b}_{ct}")
                    nc.tensor.matmul(ps2, lhsT=ST_sb, rhs=gm,
                                     start=True, stop=True)
                    nc.vector.tensor_mul(A_sb[:, ct:ct + 1], ps2[:, 1:2],
                                         nsc_sb[:, ct:ct + 1])
                    bb = small.tile([128, 1], F32, tag=f"bb{ct}",
                                    name=f"bb_{b}_{ct}")
                    nc.vector.tensor_mul(bb, ps2[:, 0:1], A_sb[:, ct:ct + 1])
                    nc.vector.tensor_sub(B_sb[:, ct:ct + 1],
                                         nbi_sb[:, ct:ct + 1], bb)
                    # hn = x*A + B (f32r out), DVE/ACT alternating
                    if ct % 2 == 0:
                        nc.scalar.activation(
                            out=hn_sb[:, ct], in_=x_sb[:, ct],
                            func=AF.Identity, scale=A_sb[:, ct:ct + 1],
                            bias=B_sb[:, ct:ct + 1],
                        )
                    else:
                        nc.vector.tensor_scalar(
                            out=hn_sb[:, ct], in0=x_sb[:, ct],
                            scalar1=A_sb[:, ct:ct + 1],
                            scalar2=B_sb[:, ct:ct + 1],
                            op0=mybir.AluOpType.mult, op1=mybir.AluOpType.add,
                        )
                st[b]["hn"] = hn_sb

            def phase_qkv(b):
                """q, k (fp8 out) in [c, n]; vT (fp8 out) in [n, c]."""
                hn_sb = st[b]["hn"]
                q_sb = big.tile([128, CT, N], F8, tag="q", bufs=2,
                                name=f"q{b}")
                k_sb = big.tile([128, CT, N], F8, tag="k", bufs=2,
                                name=f"k{b}")
                evict_i = 0
                for wname, w_sb, dst, bias_sb, use_b in (
                        ("q", wq_sb, q_sb, bqt_sb, use_bq),
                        ("k", wk_sb, k_sb, bkt_sb, use_bk)):
                    for ot in range(CT):
                        pss = [psum.tile([128, 512], F32, tag="mm", bufs=6,
                                         name=f"{wname}_ps_{b}_{ot}_{ic}")
                               for ic in range(IC)]
                        for ct in range(CT):
                            for ic in range(IC):
                                nc.tensor.matmul(
                                    pss[ic],
                                    lhsT=w_sb[:, ct, ot * 128:(ot + 1) * 128],
                                    rhs=hn_sb[:, ct, ic * 512:(ic + 1) * 512],
                                    start=(ct == 0), stop=(ct == CT - 1),
                                )
                        for ic in range(IC):
                            out = dst[:, ot, ic * 512:(ic + 1) * 512]
                            if use_b:
                                if evict_i % 2 == 0:
                                    nc.vector.tensor_scalar_add(
                                        out, pss[ic], bias_sb[:, ot:ot + 1])
                                else:
                                    nc.scalar.activation(
                                        out=out, in_=pss[ic], func=AF.Identity,
                                        bias=bias_sb[:, ot:ot + 1], scale=1.0)
                            else:
                                if evict_i % 2 == 0:
                                    nc.vector.tensor_copy(out, pss[ic])
                                else:
                                    nc.scalar.activation(
                                        out=out, in_=pss[ic], func=AF.Identity,
                                        bias=0.0, scale=1.0)
                            evict_i += 1
                vT_sb = big.tile([128, NT, 512], F8, tag="vT", bufs=2,
                                 name=f"vT{b}")
                for nt in range(NT):
                    ps = psum.tile([128, 512], F32, tag="mm", bufs=6,
                                   name=f"v_ps_{b}_{nt}")
                    for ct in range(CT):
                        nc.tensor.matmul(
                            ps,
                            lhsT=hn_sb[:, ct, nt * 128:(nt + 1) * 128],
                            rhs=wv_sb[:, ct, :],
                            start=(ct == 0), stop=(ct == CT - 1),
                        )
                    if nt % 2 == 0:
                        nc.vector.tensor_copy(vT_sb[:, nt], ps)
                    else:
                        nc.scalar.activation(out=vT_sb[:, nt], in_=ps,
                                             func=AF.Identity, bias=0.0,
                                             scale=1.0)
                st[b]["q"], st[b]["k"], st[b]["vT"] = q_sb, k_sb, vT_sb

            def phase_attn(b):
                """scores->exp (fp8), row sums, AV, all DoubleRow fp8."""
                q_sb, k_sb, vT_sb = st[b]["q"], st[b]["k"], st[b]["vT"]
                eTs = [big.tile([128, NT, 512], F8, tag="eT", bufs=4,
                                name=f"eT_{b}_{ic}") for ic in range(IC)]
                for jt in range(NT):
                    pss = [psum.tile([128, 512], F32, tag="mm", bufs=6,
                                     name=f"sc_ps_{b}_{jt}_{ic}")
                           for ic in range(IC)]
                    for cp in range(CP):
                        for ic in range(IC):
                            nc.tensor.matmul(
                                pss[ic],
                                lhsT=k_sb[:, 2 * cp:2 * cp + 2,
                                          jt * 128:(jt + 1) * 128],
                                rhs=q_sb[:, 2 * cp:2 * cp + 2,
                                         ic * 512:(ic + 1) * 512],
                                start=(cp == 0), stop=(cp == CP - 1),
                                perf_mode=DR,
                            )
                    for ic in range(IC):
                        nc.scalar.activation(
                            out=eTs[ic][:, jt], in_=pss[ic], func=AF.Exp,
                            scale=scale, bias=ebias_sb,
                        )
                # r[i] = sum_j eT[j, i] over the fp8 eT the AV GEMM sees
                rs_pss = [psum.tile([16, 512], F32, tag="small", bufs=2,
                                    name=f"rs_ps_{b}_{ic}") for ic in range(IC)]
                for jp in range(JP):
                    for ic in range(IC):
                        nc.tensor.matmul(
                            rs_pss[ic], lhsT=ones8_sb,
                            rhs=eTs[ic][:, 2 * jp:2 * jp + 2, :],
                            start=(jp == 0), stop=(jp == JP - 1),
                            perf_mode=DR,
                        )
                rsums, rinvs = [], []
                for ic in range(IC):
                    lr_sb = small.tile([1, 512], F32, tag="lnr", bufs=2,
                                       name=f"lnr_{b}_{ic}")
                    nc.scalar.activation(out=lr_sb, in_=rs_pss[ic][0:1, :], func=AF.Ln)
                    rinv_sb = small.tile([1, 512], F32R, tag="rinv", bufs=2,
                                         name=f"rinv_{b}_{ic}")
                    nc.scalar.activation(out=rinv_sb, in_=lr_sb, func=AF.Exp,
                                         scale=-1.0)
                    rinvs.append(rinv_sb)
                    if use_bv:
                        rsum_sb = small.tile([1, 512], F32R, tag="rsum",
                                             bufs=2, name=f"rsum_{b}_{ic}")
                        nc.vector.tensor_copy(rsum_sb, rs_pss[ic][0:1, :])
                        rsums.append(rsum_sb)

                avns = [big.tile([128, CT, 512], F8, tag="avn", bufs=4,
                                 name=f"avn_{b}_{ic}") for ic in range(IC)]
                av_pss = []
                bc_pss = []
                for ct in range(CT):
                    pss = [psum.tile([128, 512], F32, tag="mm", bufs=6,
                                     name=f"av_ps_{b}_{ct}_{ic}")
                           for ic in range(IC)]
                    av_pss.append(pss)
                    for jp in range(JP):
                        for ic in range(IC):
                            nc.tensor.matmul(
                                pss[ic],
                                lhsT=vT_sb[:, 2 * jp:2 * jp + 2,
                                           ct * 128:(ct + 1) * 128],
                                rhs=eTs[ic][:, 2 * jp:2 * jp + 2, :],
                                start=(jp == 0),
                                stop=(jp == JP - 1 and not use_bv),
                                perf_mode=DR,
                            )
                    if use_bv:
                        for ic in range(IC):
                            nc.tensor.matmul(
                                pss[ic],
                                lhsT=bvr_sb[0:1, ct * 128:(ct + 1) * 128],
                                rhs=rsums[ic], start=False, stop=True,
                                skip_group_check=True,
                            )
                    if ct == 0:
                        # broadcast 1/r across partitions; placed after
                        # the first AV group so the PE does not idle on
                        # the ACT ln/exp chain above.
                        for ic in range(IC):
                            bc_ps = psum.tile([128, 512], F32, tag="mm",
                                              bufs=6, name=f"bc_ps_{b}_{ic}")
                            nc.tensor.matmul(bc_ps, lhsT=ones1_sb,
                                             rhs=rinvs[ic],
                                             start=True, stop=True)
                            bc_pss.append(bc_ps)
                rinvbs = []
                for ic in range(IC):
                    rinvb_sb = small.tile([128, 512], F32, tag="rinvb", bufs=4,
                                          name=f"rinvb_{b}_{ic}")
                    nc.vector.tensor_copy(rinvb_sb, bc_pss[ic])
                    rinvbs.append(rinvb_sb)
                for ct in range(CT):
                    for ic in range(IC):
                        nc.vector.tensor_mul(avns[ic][:, ct], av_pss[ct][ic],
                                             rinvbs[ic])
                st[b]["avn"] = avns

            def phase_proj(b):
                """y = Wo av + x (+bo), residual via identity matmul."""
                x_sb = x_sbs[b]
                avns = st[b]["avn"]
                for ot in range(CT):
                    pss = [psum.tile([128, 512], F32, tag="mm", bufs=6,
                                     name=f"pr_ps_{b}_{ot}_{ic}")
                           for ic in range(IC)]
                    for cp in range(CP):
                        for ic in range(IC):
                            nc.tensor.matmul(
                                pss[ic],
                                lhsT=wo_sb[:, 2 * cp:2 * cp + 2,
                                           ot * 128:(ot + 1) * 128],
                                rhs=avns[ic][:, 2 * cp:2 * cp + 2, :],
                                start=(cp == 0), stop=False,
                                perf_mode=DR, skip_group_check=True,
                            )
                    for ic in range(IC):
                        nc.tensor.matmul(
                            pss[ic], lhsT=id_sb,
                            rhs=x_sb[:, ot, ic * 512:(ic + 1) * 512],
                            start=False, stop=(not use_bo),
                            skip_group_check=True,
                        )
                    if use_bo:
                        for ic in range(IC):
                            nc.tensor.matmul(
                                pss[ic],
                                lhsT=bor_sb[0:1, ot * 128:(ot + 1) * 128],
                                rhs=ones512_sb, start=False, stop=True,
                                skip_group_check=True,
                            )
                    y_sb = big.tile([128, N], F32, tag="y", bufs=4,
                                    name=f"y_{b}_{ot}")
                    for ic in range(IC):
                        out = y_sb[:, ic * 512:(ic + 1) * 512]
                        if ic % 2 == 0:
                            nc.scalar.activation(out=out, in_=pss[ic],
                                                 func=AF.Identity, bias=0.0,
                                                 scale=1.0)
                        else:
                            nc.vector.tensor_copy(out, pss[ic])
                    yq = nc.sync if ot % 2 == 0 else nc.gpsimd
                    yq.dma_start(out=y_d[b, :, ot, :], in_=y_sb)

            # ---- interleaved build: issue order is scheduler priority ----
            phase_norm(0)
            phase_qkv(0)
            # batch-1 x triggers early so the transfers overlap batch-0
            # compute; scalar/gpsimd queues are past their weight loads.
            x_sb1 = big.tile([128, CT, N], F32R, tag="x", bufs=2, name="x_sb1")
            x_sbs.append(x_sb1)
            for ct in range(CT):
                for h in range(2):
                    eng = nc.scalar if ct % 2 == 0 else nc.gpsimd
                    eng.dma_start(
                        out=x_sb1[:, ct, h * 512:(h + 1) * 512],
                        in_=x_d[1, :, ct, h * 512:(h + 1) * 512])
            phase_attn(0)
            phase_norm(1)
            phase_proj(0)
            phase_qkv(1)
            phase_attn(1)
            phase_proj(1)
    return nc


_CACHE = {}


def _get_nc(use_bq=False, use_bk=False, use_bv=False, use_bo=False):
    key = (use_bq, use_bk, use_bv, use_bo)
    if key not in _CACHE:
        _CACHE[key] = build_nc(*key)
    return _CACHE[key]


def prepare(x, norm_scale, norm_bias, wq, bq, wk, bk, wv, bv, wo, bo):
    """Host-side prep: returns (in_maps, flags)."""
    x = np.ascontiguousarray(np.asarray(x, dtype=np.float32))
    f32 = lambda a: np.asarray(a, dtype=np.float32)
    norm_scale, norm_bias = f32(norm_scale), f32(norm_bias)
    wq, wk, wv, wo = f32(wq), f32(wk), f32(wv), f32(wo)
    bq, bk, bv, bo = f32(bq), f32(bk), f32(bv), f32(bo)

    # [C, C] w  ->  wT[c, o] arranged [p, ct, o]
    def arr_w(w, dt):
        a = np.ascontiguousarray(w.T.reshape(CT, 128, C).transpose(1, 0, 2))
        return np.ascontiguousarray(a.astype(dt))

    # [C] vec (channel-tile major) -> [p, ct]
    def arr_c(v):
        return np.ascontiguousarray(v.reshape(CT, 128).T)

    S = np.zeros((128, GPT), np.float32)
    S[np.arange(128), np.arange(128) // GS] = 1.0
    pk1 = np.concatenate(
        [S, arr_c(norm_scale), arr_c(norm_bias), arr_c(bq), arr_c(bk)], axis=1)
    pk2 = np.concatenate(
        [np.ones(128, np.float32), np.ones(512, np.float32),
         bo.reshape(C), bv.reshape(C)]).reshape(1, -1)
    common = {
        "wqT": arr_w(wq, np.float32),
        "wkT": arr_w(wk, np.float32),
        "wvT": arr_w(wv, np.float32),
        "woT8": arr_w(wo, ml_dtypes.float8_e4m3),
        "idm": np.ascontiguousarray(np.eye(128, dtype=np.float32)),
        "pk1": np.ascontiguousarray(pk1),
        "pk2": np.ascontiguousarray(pk2),
        "ST": np.ascontiguousarray(S.T),
    }

    # x: (B, C, H, W) -> per core [NB, p, ct, n]
    xf = x.reshape(B, C, N).reshape(B, CT, 128, N).transpose(0, 2, 1, 3)
    in_maps = [
        {**common, "x": np.ascontiguousarray(xf[i * NB:(i + 1) * NB])}
        for i in range(NCORES)
    ]
    flags = (bool(np.any(bq != 0.0)), bool(np.any(bk != 0.0)),
             bool(np.any(bv != 0.0)), bool(np.any(bo != 0.0)))
    return in_maps, flags


def assemble(results):
    y = np.empty((B, C, N), np.float32)
    for i in range(NCORES):
        yc = results[i]["y"]  # [NB, 128, CT, N]
        y[i * NB:(i + 1) * NB] = (
            yc.transpose(0, 2, 1, 3).reshape(NB, C, N))
    return y.reshape(B, C, H, W)


def kernel(x, norm_scale, norm_bias, wq, bq, wk, bk, wv, bv, wo, bo):
    in_maps, flags = prepare(x, norm_scale, norm_bias, wq, bq,
                             wk, bk, wv, bv, wo, bo)
    nc = _get_nc(*flags)
    res = run_bass_kernel_spmd(nc, in_maps, list(range(NCORES)))
    return assemble(res.results)


# revision 9
# speedup vs baseline: 1.1674x; 1.0499x over previous
"""Trainium2 Bass kernel for nn_AttnBlock (B=16, C=512, H=W=32).

Strategy
--------
Data-parallel over batch: 16 batch elements / 8 NeuronCores = 2 per core.
Per batch element (C=512 channels, N=1024 pixels), all on one core:

  1. GroupNorm(32 groups) in [c, n] layout, pipelined PER CHANNEL TILE
     over a bf16 copy of x (half the DMA bytes on the critical path;
     the f32 x streams later, used only for the residual).  Each
     128-channel tile's stats (bn_stats -> group aggregation via a tiny
     0/1-indicator PE matmul -> sqrt/reciprocal -> broadcast-back
     matmul) complete as soon as that tile's DMA lands; the
     hn = x*A + B apply (bf16 out) follows immediately.
  2. q = Wq hn, k = Wk hn, vT = (Wv hn)^T -- bf16 matmuls at full PE
     rate with half the weight DMA.  All three evict to fp8e4m3.
  3. Attention in fp8 DoubleRow matmuls (2 fp8 MACs per PE cell per
     cycle): eT[j,i] = exp(kq/sqrt(C) - 2) computed directly in [j, i]
     layout (the -2 bias keeps exp <= ~125 < 240, the TRN fp8e4 max;
     softmax normalization cancels it exactly).  Row sums via a
     DoubleRow ones-vector matmul (16-wide ones: dual-fp8 LDWEIGHTS
     needs a 16B-multiple pair step); 1/r via ACT ln/exp;
     av = (vT^T eT) * (1/r) evicted to fp8.
  4. proj: y = Wo av + x with Wo in fp8 DoubleRow and the residual x
     added INTO the proj PSUM by an identity-matrix f32r matmul over
     the f32 x, so the eviction is a pure copy.

Precision (sim, scale-relative absmax vs f32 reference): 1.08e-2 vs
the 2e-2 gate.  fp8 is applied only where the softmax structure damps
it; the residual path stays f32r-exact.

DMA queues (sync / gpsimd / scalar-early, ~72 GB/s each) are packed in
first-use order; evictions are balanced across ACT/DVE/GpSimd.  The
kernel graph is built once per process and reused.
"""
import contextlib
import os
import sys

for _p in ("/opt/trn_rl_repo",):
    if _p not in sys.path and os.path.isdir(_p):
        sys.path.append(_p)

import numpy as np
import ml_dtypes

import concourse.bass as bass
import concourse.tile as tile
from concourse import mybir
from concourse.bass_utils import run_bass_kernel_spmd
from concourse.vector_clock import ScopedClock

F32 = mybir.dt.float32
F32R = mybir.dt.float32r
BF16 = mybir.dt.bfloat16
F8 = mybir.dt.float8e4
AF = mybir.ActivationFunctionType
DR = mybir.MatmulPerfMode.DoubleRow

NCORES = 8
B, C, N = 16, 512, 1024
H = W = 32
NB = B // NCORES          # batch elements per core
CT = C // 128             # channel tiles of 128
NT = N // 128             # pixel tiles of 128
IC = N // 512             # query chunks of 512
CP = CT // 2              # channel-tile pairs (DoubleRow K=256)
JP = NT // 2              # pixel-tile pairs (DoubleRow K=256)
G, GS = 32, 16            # groups, channels per group
GPT = 128 // GS           # groups per 128-channel tile
EPS = 1e-6
EXP_BIAS = 2.0            # exp(s - 2): keeps eT <= ~125 < 240 (fp8e4 max)


class _TC(tile.TileContext):
    """TileContext with multi-wait instructions split for this walrus.

    The pinned walrus accepts at most one semaphore wait per instruction
    (two for EventSemaphore).  Tile's scheduler can attach several; the
    extras are moved onto no-op carriers committed immediately before on
    the same engine, which is semantically identical (engine streams are
    sequential).
    """

    def _commit_instruction(self, inst, lazy_reg_writes: bool = True):
        si = inst.sync_info
        cap = 2 if isinstance(inst, mybir.InstEventSemaphore) else 1
        if si is not None and si.on_wait and len(si.on_wait) > cap and \
                inst.engine != mybir.EngineType.Unassigned:
            waits = list(si.on_wait)
            inst.sync_info = mybir.SyncInfo(
                on_wait=waits[:cap], on_update=list(si.on_update or [])
            )
            for w in waits[cap:]:
                nop = mybir.InstNoOp(
                    name=self.nc.get_next_instruction_name(),
                    ins=[],
                    outs=[],
                    engine=inst.engine,
                    sync_info=mybir.SyncInfo(on_wait=[w], on_update=[]),
                    bass_nofuse=True,
                )
                super()._commit_instruction(nop, lazy_reg_writes=False)
        super()._commit_instruction(inst, lazy_reg_writes)

    def _drain_and_barrier(self, tick_clock, wait_clock):
        # Collect the final-tick waits on a probe drain, then distribute
        # them across all engines (one wait per carrier instruction).
        # Each engine then signals a star-barrier semaphore; gpsimd
        # collects all signals and clears the semaphores.  This replaces
        # Tile's two EVSEM-butterfly all-engine barriers (~10us).
        nc = self.nc
        drain_inst = nc.sync.drain()
        wait_clock.add_sem_waits(
            drain_inst.ins, ScopedClock({None: tick_clock.global_clock})
        )
        si = drain_inst.ins.sync_info
        waits = list(si.on_wait) if si and si.on_wait else []
        drain_inst.ins.sync_info = mybir.SyncInfo(
            on_wait=waits[:1], on_update=[]
        )
        engines = list(nc.engines.values())
        for i, w in enumerate(waits[1:]):
            eng = engines[i % len(engines)]
            nop = eng.nop(nofuse=True)
            nop.ins.sync_info = mybir.SyncInfo(on_wait=[w], on_update=[])
        star = nc.alloc_semaphore("tile_star_barrier")
        nsig = 0
        for eng in engines:
            if eng is not nc.gpsimd:
                eng.sem_inc(star, 1)
                nsig += 1
        nc.gpsimd.wait_ge(star, nsig)
        assert self.sems is not None
        popped = nc._tile_sem_poison_stack.pop()
        assert popped is self._sem_poison
        nc.clear_and_free_semaphores(
            list(self.sems.allocated().values()) + [star])


def build_nc(use_bq: bool, use_bk: bool, use_bv: bool, use_bo: bool):
    nc = bass.Bass()

    # Per-core DRAM I/O.  x8 is the bf16 copy (groupnorm path); x is the
    # f32 original, declared f32r so the PE identity-matmul residual add
    # can read it at full rate.
    x8_d = nc.declare_dram_parameter("x8", [NB, 128, CT, N], BF16, isOutput=False)
    x_d = nc.declare_dram_parameter("x", [NB, 128, CT, N], F32R, isOutput=False)
    y_d = nc.declare_dram_parameter("y", [NB, 128, CT, N], F32, isOutput=True)
    wq_d = nc.declare_dram_parameter("wqT", [128, CT, 512], BF16, isOutput=False)
    wk_d = nc.declare_dram_parameter("wkT", [128, CT, 512], BF16, isOutput=False)
    wv_d = nc.declare_dram_parameter("wvT", [128, CT, 512], BF16, isOutput=False)
    wo_d = nc.declare_dram_parameter("woT8", [128, CT, 512], F8, isOutput=False)
    id_d = nc.declare_dram_parameter("idm", [128, 128], F32R, isOutput=False)
    # pk1 packs [S | nsc | nbi | bqt | bkt] f32 columns.
    pk1_d = nc.declare_dram_parameter("pk1", [128, GPT + 4 * CT], F32,
                                      isOutput=False)
    # pk2 packs the f32r row constants [ones1(128) | ones512(512) |
    # bor(512) | bvr(512)].
    pk2_d = nc.declare_dram_parameter("pk2", [1, 128 + 3 * 512], F32R,
                                      isOutput=False)
    ST_d = nc.declare_dram_parameter("ST", [GPT, 128], F32, isOutput=False)

    scale = float(C) ** -0.5

    with _TC(nc) as tc:
        with (
            tc.tile_pool(name="consts", bufs=1) as consts,
            tc.tile_pool(name="big", bufs=1) as big,
            tc.tile_pool(name="small", bufs=2) as small,
            tc.tile_pool(name="psum", bufs=1, space="PSUM") as psum,
        ):
            # ---- constant + weight tiles ----
            pk1_sb = consts.tile([128, GPT + 4 * CT], F32, tag="pk1")
            ST_sb = consts.tile([GPT, 128], F32, tag="ST")
            pk2_sb = consts.tile([1, 128 + 3 * 512], F32R, tag="pk2")
            id_sb = consts.tile([128, 128], F32R, tag="idm")
            wq_sb = consts.tile([128, CT, 512], BF16, tag="wq")
            wk_sb = consts.tile([128, CT, 512], BF16, tag="wk")
            wv_sb = consts.tile([128, CT, 512], BF16, tag="wv")
            wo_sb = consts.tile([128, CT, 512], F8, tag="wo")
            x8_sbs = [big.tile([128, CT, N], BF16, tag="x8", bufs=2,
                               name=f"x8_{b}") for b in range(NB)]
            x_sbs = [big.tile([128, CT, N], F32R, tag="x", bufs=2,
                              name=f"x_{b}") for b in range(NB)]

            S_sb = pk1_sb[:, 0:GPT]
            nsc_sb = pk1_sb[:, GPT:GPT + CT]
            nbi_sb = pk1_sb[:, GPT + CT:GPT + 2 * CT]
            bqt_sb = pk1_sb[:, GPT + 2 * CT:GPT + 3 * CT]
            bkt_sb = pk1_sb[:, GPT + 3 * CT:GPT + 4 * CT]
            ones1_sb = pk2_sb[:, 0:128]
            ones512_sb = pk2_sb[:, 128:640]
            bor_sb = pk2_sb[:, 640:1152]
            bvr_sb = pk2_sb[:, 1152:1664]

            # ---- DMA schedule: queues are ~72 GB/s each; floors tell
            # the scheduler (whose cost model thinks DMA is instant)
            # roughly when each trigger should issue so it keeps this
            # order and models realistic arrival for dependents.
            def dma(eng, floor, out, in_):
                with tc.tile_wait_until(floor, enable=False):
                    eng.dma_start(out=out, in_=in_)

            # scalar (early only; ACT is idle until ~6 sched-us)
            dma(nc.scalar, 0, pk1_sb, pk1_d[:, :])
            dma(nc.scalar, 0, ST_sb, ST_d[:, :])
            dma(nc.scalar, 0, pk2_sb, pk2_d[:, :])
            dma(nc.scalar, 0.0016, x8_sbs[0][:, 0, 0:512], x8_d[0, :, 0, 0:512])
            dma(nc.scalar, 0.0034, x8_sbs[0][:, 0, 512:1024],
                x8_d[0, :, 0, 512:1024])
            dma(nc.scalar, 0.0052, x8_sbs[0][:, 1, 512:1024],
                x8_d[0, :, 1, 512:1024])
            dma(nc.scalar, 0.0070, x8_sbs[0][:, 2, 0:512], x8_d[0, :, 2, 0:512])
            dma(nc.scalar, 0.0088, x8_sbs[0][:, 3, 512:1024],
                x8_d[0, :, 3, 512:1024])
            dma(nc.scalar, 0.0106, wo_sb, wo_d[:, :, :])
            dma(nc.scalar, 0.0142, id_sb, id_d[:, :])
            # sync
            dma(nc.sync, 0, wq_sb, wq_d[:, :, :])
            dma(nc.sync, 0.0078, wk_sb, wk_d[:, :, :])
            dma(nc.sync, 0.0149, x_sbs[0][:, 0:2], x_d[0, :, 0:2])
            dma(nc.sync, 0.0220, x_sbs[0][:, 2:4], x_d[0, :, 2:4])
            # gpsimd
            dma(nc.gpsimd, 0.0007, x8_sbs[0][:, 1, 0:512], x8_d[0, :, 1, 0:512])
            dma(nc.gpsimd, 0.0025, x8_sbs[0][:, 2, 512:1024],
                x8_d[0, :, 2, 512:1024])
            dma(nc.gpsimd, 0.0043, x8_sbs[0][:, 3, 0:512], x8_d[0, :, 3, 0:512])
            dma(nc.gpsimd, 0.0061, wv_sb, wv_d[:, :, :])

            # bn_stats floors: max of the two half-tile arrivals above.
            arrive_ms = {
                0: {(0, 0): 0.0034, (0, 1): 0.0052,
                    (1, 0): 0.0025, (1, 1): 0.0070,
                    (2, 0): 0.0088, (2, 1): 0.0043,
                    (3, 0): 0.0061, (3, 1): 0.0106},
                1: {(ct, h): 0.0150 + 0.0036 * ct for ct in range(CT)
                    for h in range(2)},
            }

            eps_sb = consts.tile([GPT, 1], F32, tag="eps")
            nc.vector.memset(eps_sb, EPS)
            ebias_sb = consts.tile([128, 1], F32, tag="ebias")
            nc.vector.memset(ebias_sb, -EXP_BIAS)
            # ones for the DoubleRow row-sum; 16 columns because dual-fp8
            # LDWEIGHTS needs the pair-dim step to be a multiple of 16B.
            ones8_sb = consts.tile([128, 2, 16], F8, tag="ones8")
            nc.vector.memset(ones8_sb, 1.0)
            # Warm the ACT tables used later (first use of a function
            # pays the table load) while DMAs stream.
            for wf, wname in ((AF.Sqrt, "sqw"), (AF.Exp, "exw"),
                              (AF.Ln, "lnw"), (AF.Identity, "idw")):
                wt = consts.tile([GPT, 1], F32, tag=wname)
                nc.scalar.activation(out=wt, in_=eps_sb, func=wf,
                                     bias=0.0, scale=1.0)

            # Per-batch state carried across the phase interleave below.
            st = [dict() for _ in range(NB)]

            def phase_norm(b):
                """GroupNorm pipelined per channel tile + hn apply."""
                x8_sb = x8_sbs[b]
                A_sb = small.tile([128, CT], F32, tag="A", name=f"A{b}")
                B_sb = small.tile([128, CT], F32, tag="B", name=f"B{b}")
                hn_sb = big.tile([128, CT, N], BF16, tag="hn", bufs=2,
                                 name=f"hn{b}")
                for ct in range(CT):
                    stats = small.tile([128, 2, 6], F32, tag=f"bnst{ct}",
                                       name=f"bnst_{b}_{ct}")
                    ts = small.tile([128, 2], F32, tag=f"ts{ct}",
                                    name=f"ts_{b}_{ct}")
                    mv = small.tile([128, 2], F32, tag=f"mv{ct}",
                                    name=f"mv_{b}_{ct}")
                    for h in range(2):
                        with tc.tile_wait_until(arrive_ms[b][(ct, h)],
                                                enable=False):
                            nc.vector.bn_stats(
                                out=stats[:, h],
                                in_=x8_sb[:, ct, h * 512:(h + 1) * 512],
                            )
                    nc.vector.bn_aggr(out=mv, in_=stats)
                    nc.vector.tensor_copy(ts[:, 0:1], mv[:, 0:1])
                    nc.vector.tensor_mul(ts[:, 1:2], mv[:, 0:1], mv[:, 0:1])
                    nc.vector.tensor_add(ts[:, 1:2], ts[:, 1:2], mv[:, 1:2])
                    ps = psum.tile([GPT, 2], F32, tag="mm", bufs=6,
                                   name=f"stat_ps_{b}_{ct}")
                    nc.tensor.matmul(ps, lhsT=S_sb, rhs=ts,
                                     start=True, stop=True)
                    # group mean / rstd for this tile's 8 groups
                    gm = small.tile([GPT, 2], F32, tag=f"gm{ct}",
                                    name=f"gm_{b}_{ct}")
                    nc.vector.tensor_scalar_mul(gm, ps, 1.0 / GS)
                    tmp = small.tile([GPT, 1], F32, tag=f"tmp{ct}",
                                     name=f"tmp_{b}_{ct}")
                    nc.vector.tensor_mul(tmp, gm[:, 0:1], gm[:, 0:1])
                    nc.vector.tensor_sub(gm[:, 1:2], gm[:, 1:2], tmp)
                    nc.scalar.activation(out=gm[:, 1:2], in_=gm[:, 1:2],
                                         func=AF.Sqrt, bias=eps_sb, scale=1.0)
                    nc.vector.reciprocal(gm[:, 1:2], gm[:, 1:2])
                    ps2 = psum.tile([128, 2], F32, tag="mm", bufs=6,
                                    name=f"ab_ps_{b}_{ct}")
                    nc.tensor.matmul(ps2, lhsT=ST_sb, rhs=gm,
                                     start=True, stop=True)
                    nc.vector.tensor_mul(A_sb[:, ct:ct + 1], ps2[:, 1:2],
                                         nsc_sb[:, ct:ct + 1])
                    bb = small.tile([128, 1], F32, tag=f"bb{ct}",
                                    name=f"bb_{b}_{ct}")
                    nc.vector.tensor_mul(bb, ps2[:, 0:1], A_sb[:, ct:ct + 1])
                    nc.vector.tensor_sub(B_sb[:, ct:ct + 1],
                                         nbi_sb[:, ct:ct + 1], bb)
                    # hn = x*A + B (bf16 out), DVE / GpSimd alternating
                    eng = nc.vector if ct % 2 == 0 else nc.gpsimd
                    eng.tensor_scalar(
                        out=hn_sb[:, ct], in0=x8_sb[:, ct],
                        scalar1=A_sb[:, ct:ct + 1],
                        scalar2=B_sb[:, ct:ct + 1],
                        op0=mybir.AluOpType.mult, op1=mybir.AluOpType.add,
                    )
                st[b]["hn"] = hn_sb

            def phase_qkv(b):
                """q, k (fp8 out) in [c, n]; vT (fp8 out) in [n, c]."""
                hn_sb = st[b]["hn"]
                q_sb = big.tile([128, CT, N], F8, tag="q", bufs=2,
                                name=f"q{b}")
                k_sb = big.tile([128, CT, N], F8, tag="k", bufs=2,
                                name=f"k{b}")
                evict_i = 0
                for wname, w_sb, dst, bias_sb, use_b in (
                        ("q", wq_sb, q_sb, bqt_sb, use_bq),
                        ("k", wk_sb, k_sb, bkt_sb, use_bk)):
                    for ot in range(CT):
                        pss = [psum.tile([128, 512], F32, tag="mm", bufs=6,
                                         name=f"{wname}_ps_{b}_{ot}_{ic}")
                               for ic in range(IC)]
                        for ct in range(CT):
                            for ic in range(IC):
                                nc.tensor.matmul(
                                    pss[ic],
                                    lhsT=w_sb[:, ct, ot * 128:(ot + 1) * 128],
                                    rhs=hn_sb[:, ct, ic * 512:(ic + 1) * 512],
                                    start=(ct == 0), stop=(ct == CT - 1),
                                )
                        for ic in range(IC):
                            out = dst[:, ot, ic * 512:(ic + 1) * 512]
                            if use_b:
                                if evict_i % 2 == 0:
                                    nc.vector.tensor_scalar_add(
                                        out, pss[ic], bias_sb[:, ot:ot + 1])
                                else:
                                    nc.scalar.activation(
                                        out=out, in_=pss[ic], func=AF.Identity,
                                        bias=bias_sb[:, ot:ot + 1], scale=1.0)
                            else:
                                if evict_i % 2 == 0:
                                    nc.vector.tensor_copy(out, pss[ic])
                                else:
                                    nc.scalar.activation(
                                        out=out, in_=pss[ic], func=AF.Identity,
                                        bias=0.0, scale=1.0)
                            evict_i += 1
                vT_sb = big.tile([128, NT, 512], F8, tag="vT", bufs=2,
                                 name=f"vT{b}")
                for nt in range(NT):
                    ps = psum.tile([128, 512], F32, tag="mm", bufs=6,
                                   name=f"v_ps_{b}_{nt}")
                    for ct in range(CT):
                        nc.tensor.matmul(
                            ps,
                            lhsT=hn_sb[:, ct, nt * 128:(nt + 1) * 128],
                            rhs=wv_sb[:, ct, :],
                            start=(ct == 0), stop=(ct == CT - 1),
                        )
                    if nt % 2 == 0:
                        nc.vector.tensor_copy(vT_sb[:, nt], ps)
                    else:
                        nc.scalar.activation(out=vT_sb[:, nt], in_=ps,
                                             func=AF.Identity, bias=0.0,
                                             scale=1.0)
                st[b]["q"], st[b]["k"], st[b]["vT"] = q_sb, k_sb, vT_sb

            def phase_attn(b):
                """scores->exp (fp8), row sums, AV, all DoubleRow fp8."""
                q_sb, k_sb, vT_sb = st[b]["q"], st[b]["k"], st[b]["vT"]
                eTs = [big.tile([128, NT, 512], F8, tag="eT", bufs=4,
                                name=f"eT_{b}_{ic}") for ic in range(IC)]
                for jt in range(NT):
                    pss = [psum.tile([128, 512], F32, tag="mm", bufs=6,
                                     name=f"sc_ps_{b}_{jt}_{ic}")
                           for ic in range(IC)]
                    for cp in range(CP):
                        for ic in range(IC):
                            nc.tensor.matmul(
                                pss[ic],
                                lhsT=k_sb[:, 2 * cp:2 * cp + 2,
                                          jt * 128:(jt + 1) * 128],
                                rhs=q_sb[:, 2 * cp:2 * cp + 2,
                                         ic * 512:(ic + 1) * 512],
                                start=(cp == 0), stop=(cp == CP - 1),
                                perf_mode=DR,
                            )
                    for ic in range(IC):
                        nc.scalar.activation(
                            out=eTs[ic][:, jt], in_=pss[ic], func=AF.Exp,
                            scale=scale, bias=ebias_sb,
                        )
                # r[i] = sum_j eT[j, i] over the fp8 eT the AV GEMM sees
                rs_pss = [psum.tile([16, 512], F32, tag="small", bufs=2,
                                    name=f"rs_ps_{b}_{ic}") for ic in range(IC)]
                for jp in range(JP):
                    for ic in range(IC):
                        nc.tensor.matmul(
                            rs_pss[ic], lhsT=ones8_sb,
                            rhs=eTs[ic][:, 2 * jp:2 * jp + 2, :],
                            start=(jp == 0), stop=(jp == JP - 1),
                            perf_mode=DR,
                        )
                rsums, rinvs = [], []
                for ic in range(IC):
                    lr_sb = small.tile([1, 512], F32, tag="lnr", bufs=2,
                                       name=f"lnr_{b}_{ic}")
                    nc.scalar.activation(out=lr_sb, in_=rs_pss[ic][0:1, :],
                                         func=AF.Ln)
                    rinv_sb = small.tile([1, 512], F32R, tag="rinv", bufs=2,
                                         name=f"rinv_{b}_{ic}")
                    nc.scalar.activation(out=rinv_sb, in_=lr_sb, func=AF.Exp,
                                         scale=-1.0)
                    rinvs.append(rinv_sb)
                    if use_bv:
                        rsum_sb = small.tile([1, 512], F32R, tag="rsum",
                                             bufs=2, name=f"rsum_{b}_{ic}")
                        nc.vector.tensor_copy(rsum_sb, rs_pss[ic][0:1, :])
                        rsums.append(rsum_sb)

                avns = [big.tile([128, CT, 512], F8, tag="avn", bufs=4,
                                 name=f"avn_{b}_{ic}") for ic in range(IC)]
                av_pss = []
                bc_pss = []
                for ct in range(CT):
                    pss = [psum.tile([128, 512], F32, tag="mm", bufs=6,
                                     name=f"av_ps_{b}_{ct}_{ic}")
                           for ic in range(IC)]
                    av_pss.append(pss)
                    for jp in range(JP):
                        for ic in range(IC):
                            nc.tensor.matmul(
                                pss[ic],
                                lhsT=vT_sb[:, 2 * jp:2 * jp + 2,
                                           ct * 128:(ct + 1) * 128],
                                rhs=eTs[ic][:, 2 * jp:2 * jp + 2, :],
                                start=(jp == 0),
                                stop=(jp == JP - 1 and not use_bv),
                                perf_mode=DR,
                            )
                    if use_bv:
                        for ic in range(IC):
                            nc.tensor.matmul(
                                pss[ic],
                                lhsT=bvr_sb[0:1, ct * 128:(ct + 1) * 128],
                                rhs=rsums[ic], start=False, stop=True,
                                skip_group_check=True,
                            )
                    if ct == 0:
                        # broadcast 1/r across partitions; placed after
                        # the first AV group so the PE does not idle on
                        # the ACT ln/exp chain above.
                        for ic in range(IC):
                            bc_ps = psum.tile([128, 512], F32, tag="mm",
                                              bufs=6, name=f"bc_ps_{b}_{ic}")
                            nc.tensor.matmul(bc_ps, lhsT=ones1_sb,
                                             rhs=rinvs[ic],
                                             start=True, stop=True)
                            bc_pss.append(bc_ps)
                rinvbs = []
                for ic in range(IC):
                    rinvb_sb = small.tile([128, 512], F32, tag="rinvb", bufs=4,
                                          name=f"rinvb_{b}_{ic}")
                    nc.vector.tensor_copy(rinvb_sb, bc_pss[ic])
                    rinvbs.append(rinvb_sb)
                for ct in range(CT):
                    for ic in range(IC):
                        nc.vector.tensor_mul(avns[ic][:, ct], av_pss[ct][ic],
                                             rinvbs[ic])
                st[b]["avn"] = avns

            def phase_proj(b):
                """y = Wo av + x (+bo), residual via identity matmul."""
                x_sb = x_sbs[b]
                avns = st[b]["avn"]
                for ot in range(CT):
                    pss = [psum.tile([128, 512], F32, tag="mm", bufs=6,
                                     name=f"pr_ps_{b}_{ot}_{ic}")
                           for ic in range(IC)]
                    for cp in range(CP):
                        for ic in range(IC):
                            nc.tensor.matmul(
                                pss[ic],
                                lhsT=wo_sb[:, 2 * cp:2 * cp + 2,
                                           ot * 128:(ot + 1) * 128],
                                rhs=avns[ic][:, 2 * cp:2 * cp + 2, :],
                                start=(cp == 0), stop=False,
                                perf_mode=DR, skip_group_check=True,
                            )
                    for ic in range(IC):
                        nc.tensor.matmul(
                            pss[ic], lhsT=id_sb,
                            rhs=x_sb[:, ot, ic * 512:(ic + 1) * 512],
                            start=False, stop=(not use_bo),
                            skip_group_check=True,
                        )
                    if use_bo:
                        for ic in range(IC):
                            nc.tensor.matmul(
                                pss[ic],
                                lhsT=bor_sb[0:1, ot * 128:(ot + 1) * 128],
                                rhs=ones512_sb, start=False, stop=True,
                                skip_group_check=True,
                            )
                    y_sb = big.tile([128, N], F32, tag="y", bufs=4,
                                    name=f"y_{b}_{ot}")
                    for ic in range(IC):
                        out = y_sb[:, ic * 512:(ic + 1) * 512]
                        if ic % 2 == 0:
                            nc.scalar.activation(out=out, in_=pss[ic],
                                                 func=AF.Identity, bias=0.0,
                                                 scale=1.0)
                        else:
                            nc.vector.tensor_copy(out, pss[ic])
                    yq = nc.sync if ot % 2 == 0 else nc.gpsimd
                    yq.dma_start(out=y_d[b, :, ot, :], in_=y_sb)

            # ---- interleaved build: issue order is scheduler priority ----
            phase_norm(0)
            phase_qkv(0)
            # batch-1 x8 on the scalar queue (free after wo/id); floors
            # keep these triggers behind batch-0's critical transfers.
            for ct in range(CT):
                dma(nc.scalar, 0.0150 + 0.0036 * ct,
                    x8_sbs[1][:, ct], x8_d[1, :, ct])
            phase_attn(0)
            phase_norm(1)
            # batch-1 f32 x for the residual, needed by phase_proj(1).
            dma(nc.scalar, 0.0294, x_sbs[1][:, 0:2], x_d[1, :, 0:2])
            dma(nc.scalar, 0.0365, x_sbs[1][:, 2:4], x_d[1, :, 2:4])
            phase_proj(0)
            phase_qkv(1)
            phase_attn(1)
            phase_proj(1)
    return nc


_CACHE = {}


def _get_nc(use_bq=False, use_bk=False, use_bv=False, use_bo=False):
    key = (use_bq, use_bk, use_bv, use_bo)
    if key not in _CACHE:
        _CACHE[key] = build_nc(*key)
    return _CACHE[key]


def prepare(x, norm_scale, norm_bias, wq, bq, wk, bk, wv, bv, wo, bo):
    """Host-side prep: returns (in_maps, flags)."""
    x = np.ascontiguousarray(np.asarray(x, dtype=np.float32))
    f32 = lambda a: np.asarray(a, dtype=np.float32)
    norm_scale, norm_bias = f32(norm_scale), f32(norm_bias)
    wq, wk, wv, wo = f32(wq), f32(wk), f32(wv), f32(wo)
    bq, bk, bv, bo = f32(bq), f32(bk), f32(bv), f32(bo)

    # [C, C] w  ->  wT[c, o] arranged [p, ct, o]
    def arr_w(w, dt):
        a = np.ascontiguousarray(w.T.reshape(CT, 128, C).transpose(1, 0, 2))
        return np.ascontiguousarray(a.astype(dt))

    # [C] vec (channel-tile major) -> [p, ct]
    def arr_c(v):
        return np.ascontiguousarray(v.reshape(CT, 128).T)

    S = np.zeros((128, GPT), np.float32)
    S[np.arange(128), np.arange(128) // GS] = 1.0
    pk1 = np.concatenate(
        [S, arr_c(norm_scale), arr_c(norm_bias), arr_c(bq), arr_c(bk)], axis=1)
    pk2 = np.concatenate(
        [np.ones(128, np.float32), np.ones(512, np.float32),
         bo.reshape(C), bv.reshape(C)]).reshape(1, -1)
    common = {
        "wqT": arr_w(wq, ml_dtypes.bfloat16),
        "wkT": arr_w(wk, ml_dtypes.bfloat16),
        "wvT": arr_w(wv, ml_dtypes.bfloat16),
        "woT8": arr_w(wo, ml_dtypes.float8_e4m3),
        "idm": np.ascontiguousarray(np.eye(128, dtype=np.float32)),
        "pk1": np.ascontiguousarray(pk1),
        "pk2": np.ascontiguousarray(pk2),
        "ST": np.ascontiguousarray(S.T),
    }

    # x: (B, C, H, W) -> per core [NB, p, ct, n]
    xf = x.reshape(B, C, N).reshape(B, CT, 128, N).transpose(0, 2, 1, 3)
    x8f = np.ascontiguousarray(xf.astype(ml_dtypes.bfloat16))
    in_maps = [
        {**common,
         "x": np.ascontiguousarray(xf[i * NB:(i + 1) * NB]),
         "x8": np.ascontiguousarray(x8f[i * NB:(i + 1) * NB])}
        for i in range(NCORES)
    ]
    flags = (bool(np.any(bq != 0.0)), bool(np.any(bk != 0.0)),
             bool(np.any(bv != 0.0)), bool(np.any(bo != 0.0)))
    return in_maps, flags


def assemble(results):
    y = np.empty((B, C, N), np.float32)
    for i in range(NCORES):
        yc = results[i]["y"]  # [NB, 128, CT, N]
        y[i * NB:(i + 1) * NB] = (
            yc.transpose(0, 2, 1, 3).reshape(NB, C, N))
    return y.reshape(B, C, H, W)


def kernel(x, norm_scale, norm_bias, wq, bq, wk, bk, wv, bv, wo, bo):
    in_maps, flags = prepare(x, norm_scale, norm_bias, wq, bq,
                             wk, bk, wv, bv, wo, bo)
    nc = _get_nc(*flags)
    res = run_bass_kernel_spmd(nc, in_maps, list(range(NCORES)))
    return assemble(res.results)


# revision 10
# speedup vs baseline: 1.1931x; 1.0221x over previous
"""Trainium2 Bass kernel for nn_AttnBlock (B=16, C=512, H=W=32).

Strategy
--------
Data-parallel over batch: 16 batch elements / 8 NeuronCores = 2 per core.
Per batch element (C=512 channels, N=1024 pixels), all on one core:

  1. GroupNorm(32 groups) in [c, n] layout, pipelined PER CHANNEL TILE
     over a bf16 copy of x (half the DMA bytes on the critical path;
     the f32 x streams later, used only for the residual).  Each
     128-channel tile's stats (bn_stats -> group aggregation via a tiny
     0/1-indicator PE matmul -> sqrt/reciprocal -> broadcast-back
     matmul) complete as soon as that tile's DMA lands; the
     hn = x*A + B apply (bf16 out) follows immediately.
  2. q = Wq hn, k = Wk hn, vT = (Wv hn)^T -- bf16 matmuls at full PE
     rate with half the weight DMA.  All three evict to fp8e4m3.
  3. Attention in fp8 DoubleRow matmuls (2 fp8 MACs per PE cell per
     cycle): eT[j,i] = exp(kq/sqrt(C) - 2) computed directly in [j, i]
     layout (the -2 bias keeps exp <= ~125 < 240, the TRN fp8e4 max;
     softmax normalization cancels it exactly).  Row sums via a
     DoubleRow ones-vector matmul (16-wide ones: dual-fp8 LDWEIGHTS
     needs a 16B-multiple pair step); 1/r via ACT ln/exp;
     av = (vT^T eT) * (1/r) evicted to fp8.
  4. proj: y = Wo av + x with Wo in fp8 DoubleRow and the residual x
     added INTO the proj PSUM by an identity-matrix f32r matmul over
     the f32 x, so the eviction is a pure copy.

Precision (sim, scale-relative absmax vs f32 reference): 1.08e-2 vs
the 2e-2 gate.  fp8 is applied only where the softmax structure damps
it; the residual path stays f32r-exact.

DMA queues (sync / gpsimd / scalar-early, ~72 GB/s each) are packed in
first-use order; evictions are balanced across ACT/DVE/GpSimd.  The
kernel graph is built once per process and reused.
"""
import contextlib
import os
import sys

for _p in ("/opt/trn_rl_repo",):
    if _p not in sys.path and os.path.isdir(_p):
        sys.path.append(_p)

import numpy as np
import ml_dtypes

import concourse.bass as bass
import concourse.tile as tile
from concourse import mybir
from concourse.bass_utils import run_bass_kernel_spmd
from concourse.vector_clock import ScopedClock

F32 = mybir.dt.float32
F32R = mybir.dt.float32r
BF16 = mybir.dt.bfloat16
F8 = mybir.dt.float8e4
AF = mybir.ActivationFunctionType
DR = mybir.MatmulPerfMode.DoubleRow

NCORES = 8
B, C, N = 16, 512, 1024
H = W = 32
NB = B // NCORES          # batch elements per core
CT = C // 128             # channel tiles of 128
NT = N // 128             # pixel tiles of 128
IC = N // 512             # query chunks of 512
CP = CT // 2              # channel-tile pairs (DoubleRow K=256)
JP = NT // 2              # pixel-tile pairs (DoubleRow K=256)
G, GS = 32, 16            # groups, channels per group
GPT = 128 // GS           # groups per 128-channel tile
EPS = 1e-6
EXP_BIAS = 2.0            # exp(s - 2): keeps eT <= ~125 < 240 (fp8e4 max)


class _TC(tile.TileContext):
    """TileContext with multi-wait instructions split for this walrus.

    The pinned walrus accepts at most one semaphore wait per instruction
    (two for EventSemaphore).  Tile's scheduler can attach several; the
    extras are moved onto no-op carriers committed immediately before on
    the same engine, which is semantically identical (engine streams are
    sequential).
    """

    def _commit_instruction(self, inst, lazy_reg_writes: bool = True):
        si = inst.sync_info
        cap = 2 if isinstance(inst, mybir.InstEventSemaphore) else 1
        if si is not None and si.on_wait and len(si.on_wait) > cap and \
                inst.engine != mybir.EngineType.Unassigned:
            waits = list(si.on_wait)
            inst.sync_info = mybir.SyncInfo(
                on_wait=waits[:cap], on_update=list(si.on_update or [])
            )
            for w in waits[cap:]:
                nop = mybir.InstNoOp(
                    name=self.nc.get_next_instruction_name(),
                    ins=[],
                    outs=[],
                    engine=inst.engine,
                    sync_info=mybir.SyncInfo(on_wait=[w], on_update=[]),
                    bass_nofuse=True,
                )
                super()._commit_instruction(nop, lazy_reg_writes=False)
        super()._commit_instruction(inst, lazy_reg_writes)

    def _drain_and_barrier(self, tick_clock, wait_clock):
        # Collect the final-tick waits on a probe drain, then distribute
        # them across all engines (one wait per carrier instruction).
        # Each engine then signals a star-barrier semaphore; gpsimd
        # collects all signals and clears the semaphores.  This replaces
        # Tile's two EVSEM-butterfly all-engine barriers (~10us).
        nc = self.nc
        drain_inst = nc.sync.drain()
        wait_clock.add_sem_waits(
            drain_inst.ins, ScopedClock({None: tick_clock.global_clock})
        )
        si = drain_inst.ins.sync_info
        waits = list(si.on_wait) if si and si.on_wait else []
        drain_inst.ins.sync_info = mybir.SyncInfo(
            on_wait=waits[:1], on_update=[]
        )
        engines = list(nc.engines.values())
        for i, w in enumerate(waits[1:]):
            eng = engines[i % len(engines)]
            nop = eng.nop(nofuse=True)
            nop.ins.sync_info = mybir.SyncInfo(on_wait=[w], on_update=[])
        star = nc.alloc_semaphore("tile_star_barrier")
        nsig = 0
        for eng in engines:
            if eng is not nc.gpsimd:
                eng.sem_inc(star, 1)
                nsig += 1
        nc.gpsimd.wait_ge(star, nsig)
        assert self.sems is not None
        popped = nc._tile_sem_poison_stack.pop()
        assert popped is self._sem_poison
        nc.clear_and_free_semaphores(
            list(self.sems.allocated().values()) + [star])


def build_nc(use_bq: bool, use_bk: bool, use_bv: bool, use_bo: bool):
    nc = bass.Bass()

    # Per-core DRAM I/O.  x8 is the bf16 copy (groupnorm path); x is the
    # f32 original, declared f32r so the PE identity-matmul residual add
    # can read it at full rate.
    x8_d = nc.declare_dram_parameter("x8", [NB, 128, CT, N], BF16, isOutput=False)
    x_d = nc.declare_dram_parameter("x", [NB, 128, CT, N], F32R, isOutput=False)
    y_d = nc.declare_dram_parameter("y", [NB, 128, CT, N], F32, isOutput=True)
    wq_d = nc.declare_dram_parameter("wqT", [128, CT, 512], BF16, isOutput=False)
    wk_d = nc.declare_dram_parameter("wkT", [128, CT, 512], BF16, isOutput=False)
    wv_d = nc.declare_dram_parameter("wvT", [128, CT, 512], BF16, isOutput=False)
    wo_d = nc.declare_dram_parameter("woT8", [128, CT, 512], F8, isOutput=False)
    id_d = nc.declare_dram_parameter("idm", [128, 128], F32R, isOutput=False)
    # pk1 packs [S | nsc | nbi | bqt | bkt] f32 columns.
    pk1_d = nc.declare_dram_parameter("pk1", [128, GPT + 4 * CT], F32,
                                      isOutput=False)
    # pk2 packs the f32r row constants [ones1(128) | ones512(512) |
    # bor(512) | bvr(512)].
    pk2_d = nc.declare_dram_parameter("pk2", [1, 128 + 3 * 512], F32R,
                                      isOutput=False)
    ST_d = nc.declare_dram_parameter("ST", [GPT, 128], F32, isOutput=False)

    scale = float(C) ** -0.5

    with _TC(nc) as tc:
        with (
            tc.tile_pool(name="consts", bufs=1) as consts,
            tc.tile_pool(name="big", bufs=1) as big,
            tc.tile_pool(name="small", bufs=2) as small,
            tc.tile_pool(name="psum", bufs=1, space="PSUM") as psum,
        ):
            # ---- constant + weight tiles ----
            pk1_sb = consts.tile([128, GPT + 4 * CT], F32, tag="pk1")
            ST_sb = consts.tile([GPT, 128], F32, tag="ST")
            pk2_sb = consts.tile([1, 128 + 3 * 512], F32R, tag="pk2")
            id_sb = consts.tile([128, 128], F32R, tag="idm")
            wq_sb = consts.tile([128, CT, 512], BF16, tag="wq")
            wk_sb = consts.tile([128, CT, 512], BF16, tag="wk")
            wv_sb = consts.tile([128, CT, 512], BF16, tag="wv")
            wo_sb = consts.tile([128, CT, 512], F8, tag="wo")
            x8_sbs = [big.tile([128, CT, N], BF16, tag="x8", bufs=2,
                               name=f"x8_{b}") for b in range(NB)]
            x_sbs = [big.tile([128, CT, N], F32R, tag="x", bufs=2,
                              name=f"x_{b}") for b in range(NB)]

            S_sb = pk1_sb[:, 0:GPT]
            nsc_sb = pk1_sb[:, GPT:GPT + CT]
            nbi_sb = pk1_sb[:, GPT + CT:GPT + 2 * CT]
            bqt_sb = pk1_sb[:, GPT + 2 * CT:GPT + 3 * CT]
            bkt_sb = pk1_sb[:, GPT + 3 * CT:GPT + 4 * CT]
            ones1_sb = pk2_sb[:, 0:128]
            ones512_sb = pk2_sb[:, 128:640]
            bor_sb = pk2_sb[:, 640:1152]
            bvr_sb = pk2_sb[:, 1152:1664]

            # ---- DMA schedule.  The 16 SDMA engines share ~360 GB/s and
            # run all triggered transfers CONCURRENTLY, so late transfers
            # must not be triggered early or they steal bandwidth from the
            # critical batch-0 x8 tiles.  Triggers are staged: stage A
            # fires immediately; later stages sit behind probe DMAs (or
            # compute) in the same engine stream, so they fire only once
            # the earlier stage's data has LANDED.  Floors (scheduler
            # hints) keep the modeled order consistent.
            def dma(eng, floor, out, in_):
                with tc.tile_wait_until(floor, enable=False):
                    eng.dma_start(out=out, in_=in_)

            probes = consts.tile([1, 16], BF16, tag="probe")
            # stage A: pk + batch-0 x8 + first half of wq (~1.3 MB)
            dma(nc.scalar, 0, pk1_sb, pk1_d[:, :])
            dma(nc.scalar, 0, ST_sb, ST_d[:, :])
            dma(nc.scalar, 0, pk2_sb, pk2_d[:, :])
            dma(nc.sync, 0, x8_sbs[0][:, 0], x8_d[0, :, 0])
            dma(nc.scalar, 0.0005, x8_sbs[0][:, 1], x8_d[0, :, 1])
            dma(nc.gpsimd, 0, x8_sbs[0][:, 2], x8_d[0, :, 2])
            dma(nc.sync, 0.0007, x8_sbs[0][:, 3], x8_d[0, :, 3])
            dma(nc.gpsimd, 0.0007, wq_sb[:, 0:2], wq_d[:, 0:2])
            # stage B, gated on all of batch-0 x8 having landed: rest of
            # the weights (~1.5 MB).
            with tc.tile_wait_until(0.0045, enable=False):
                nc.sync.dma_start(out=probes[:, 0:4],
                                  in_=x8_sbs[0][0:1, :, 1023:1024])
                nc.gpsimd.dma_start(out=probes[:, 4:8],
                                    in_=x8_sbs[0][0:1, :, 1022:1023])
            dma(nc.sync, 0.0050, wq_sb[:, 2:4], wq_d[:, 2:4])
            dma(nc.sync, 0.0052, wk_sb, wk_d[:, :, :])
            dma(nc.gpsimd, 0.0050, wv_sb, wv_d[:, :, :])
            dma(nc.gpsimd, 0.0054, wo_sb, wo_d[:, :, :])
            dma(nc.gpsimd, 0.0056, id_sb, id_d[:, :])
            # stage C (batch-1 x8 + batch-0 f32 x) is issued after
            # phase_qkv(0) below, behind a probe on batch-0's hn.

            # bn_stats floors: stage-A arrival estimates.
            arrive_ms = {
                0: {(ct, h): [0.0038, 0.0042, 0.0040, 0.0044][ct]
                    for ct in range(CT) for h in range(2)},
                1: {(ct, h): 0.0100 + 0.0012 * ct for ct in range(CT)
                    for h in range(2)},
            }

            eps_sb = consts.tile([GPT, 1], F32, tag="eps")
            nc.vector.memset(eps_sb, EPS)
            ebias_sb = consts.tile([128, 1], F32, tag="ebias")
            nc.vector.memset(ebias_sb, -EXP_BIAS)
            # ones for the DoubleRow row-sum; 16 columns because dual-fp8
            # LDWEIGHTS needs the pair-dim step to be a multiple of 16B.
            ones8_sb = consts.tile([128, 2, 16], F8, tag="ones8")
            nc.vector.memset(ones8_sb, 1.0)
            # Warm the ACT tables used later (first use of a function
            # pays the table load) while DMAs stream.
            for wf, wname in ((AF.Sqrt, "sqw"), (AF.Exp, "exw"),
                              (AF.Ln, "lnw"), (AF.Identity, "idw")):
                wt = consts.tile([GPT, 1], F32, tag=wname)
                nc.scalar.activation(out=wt, in_=eps_sb, func=wf,
                                     bias=0.0, scale=1.0)

            # Per-batch state carried across the phase interleave below.
            st = [dict() for _ in range(NB)]

            def phase_norm(b):
                """GroupNorm pipelined per channel tile + hn apply."""
                x8_sb = x8_sbs[b]
                A_sb = small.tile([128, CT], F32, tag="A", name=f"A{b}")
                B_sb = small.tile([128, CT], F32, tag="B", name=f"B{b}")
                hn_sb = big.tile([128, CT, N], BF16, tag="hn", bufs=2,
                                 name=f"hn{b}")
                for ct in range(CT):
                    stats = small.tile([128, 2, 6], F32, tag=f"bnst{ct}",
                                       name=f"bnst_{b}_{ct}")
                    ts = small.tile([128, 2], F32, tag=f"ts{ct}",
                                    name=f"ts_{b}_{ct}")
                    mv = small.tile([128, 2], F32, tag=f"mv{ct}",
                                    name=f"mv_{b}_{ct}")
                    for h in range(2):
                        with tc.tile_wait_until(arrive_ms[b][(ct, h)],
                                                enable=False):
                            nc.vector.bn_stats(
                                out=stats[:, h],
                                in_=x8_sb[:, ct, h * 512:(h + 1) * 512],
                            )
                    nc.vector.bn_aggr(out=mv, in_=stats)
                    nc.vector.tensor_copy(ts[:, 0:1], mv[:, 0:1])
                    nc.vector.tensor_mul(ts[:, 1:2], mv[:, 0:1], mv[:, 0:1])
                    nc.vector.tensor_add(ts[:, 1:2], ts[:, 1:2], mv[:, 1:2])
                    ps = psum.tile([GPT, 2], F32, tag="mm", bufs=6,
                                   name=f"stat_ps_{b}_{ct}")
                    nc.tensor.matmul(ps, lhsT=S_sb, rhs=ts,
                                     start=True, stop=True)
                    # group mean / rstd for this tile's 8 groups
                    gm = small.tile([GPT, 2], F32, tag=f"gm{ct}",
                                    name=f"gm_{b}_{ct}")
                    nc.vector.tensor_scalar_mul(gm, ps, 1.0 / GS)
                    tmp = small.tile([GPT, 1], F32, tag=f"tmp{ct}",
                                     name=f"tmp_{b}_{ct}")
                    nc.vector.tensor_mul(tmp, gm[:, 0:1], gm[:, 0:1])
                    nc.vector.tensor_sub(gm[:, 1:2], gm[:, 1:2], tmp)
                    nc.scalar.activation(out=gm[:, 1:2], in_=gm[:, 1:2],
                                         func=AF.Sqrt, bias=eps_sb, scale=1.0)
                    nc.vector.reciprocal(gm[:, 1:2], gm[:, 1:2])
                    ps2 = psum.tile([128, 2], F32, tag="mm", bufs=6,
                                    name=f"ab_ps_{b}_{ct}")
                    nc.tensor.matmul(ps2, lhsT=ST_sb, rhs=gm,
                                     start=True, stop=True)
                    nc.vector.tensor_mul(A_sb[:, ct:ct + 1], ps2[:, 1:2],
                                         nsc_sb[:, ct:ct + 1])
                    bb = small.tile([128, 1], F32, tag=f"bb{ct}",
                                    name=f"bb_{b}_{ct}")
                    nc.vector.tensor_mul(bb, ps2[:, 0:1], A_sb[:, ct:ct + 1])
                    nc.vector.tensor_sub(B_sb[:, ct:ct + 1],
                                         nbi_sb[:, ct:ct + 1], bb)
                    # hn = x*A + B (bf16 out).  Early-arriving tiles go
                    # to GpSimd (1.2us but off the critical path); the
                    # last tiles take DVE's faster 525ns path.
                    eng = nc.gpsimd if ct < 2 else nc.vector
                    eng.tensor_scalar(
                        out=hn_sb[:, ct], in0=x8_sb[:, ct],
                        scalar1=A_sb[:, ct:ct + 1],
                        scalar2=B_sb[:, ct:ct + 1],
                        op0=mybir.AluOpType.mult, op1=mybir.AluOpType.add,
                    )
                st[b]["hn"] = hn_sb

            def phase_qkv(b):
                """q, k (fp8 out) in [c, n]; vT (fp8 out) in [n, c]."""
                hn_sb = st[b]["hn"]
                q_sb = big.tile([128, CT, N], F8, tag="q", bufs=2,
                                name=f"q{b}")
                k_sb = big.tile([128, CT, N], F8, tag="k", bufs=2,
                                name=f"k{b}")
                evict_i = 0
                for wname, w_sb, dst, bias_sb, use_b in (
                        ("q", wq_sb, q_sb, bqt_sb, use_bq),
                        ("k", wk_sb, k_sb, bkt_sb, use_bk)):
                    for ot in range(CT):
                        pss = [psum.tile([128, 512], F32, tag="mm", bufs=6,
                                         name=f"{wname}_ps_{b}_{ot}_{ic}")
                               for ic in range(IC)]
                        for ct in range(CT):
                            for ic in range(IC):
                                nc.tensor.matmul(
                                    pss[ic],
                                    lhsT=w_sb[:, ct, ot * 128:(ot + 1) * 128],
                                    rhs=hn_sb[:, ct, ic * 512:(ic + 1) * 512],
                                    start=(ct == 0), stop=(ct == CT - 1),
                                )
                        for ic in range(IC):
                            out = dst[:, ot, ic * 512:(ic + 1) * 512]
                            if use_b:
                                if evict_i % 2 == 0:
                                    nc.vector.tensor_scalar_add(
                                        out, pss[ic], bias_sb[:, ot:ot + 1])
                                else:
                                    nc.scalar.activation(
                                        out=out, in_=pss[ic], func=AF.Identity,
                                        bias=bias_sb[:, ot:ot + 1], scale=1.0)
                            else:
                                if evict_i % 2 == 0:
                                    nc.vector.tensor_copy(out, pss[ic])
                                else:
                                    nc.scalar.activation(
                                        out=out, in_=pss[ic], func=AF.Identity,
                                        bias=0.0, scale=1.0)
                            evict_i += 1
                vT_sb = big.tile([128, NT, 512], F8, tag="vT", bufs=2,
                                 name=f"vT{b}")
                for nt in range(NT):
                    ps = psum.tile([128, 512], F32, tag="mm", bufs=6,
                                   name=f"v_ps_{b}_{nt}")
                    for ct in range(CT):
                        nc.tensor.matmul(
                            ps,
                            lhsT=hn_sb[:, ct, nt * 128:(nt + 1) * 128],
                            rhs=wv_sb[:, ct, :],
                            start=(ct == 0), stop=(ct == CT - 1),
                        )
                    if nt % 2 == 0:
                        nc.vector.tensor_copy(vT_sb[:, nt], ps)
                    else:
                        nc.scalar.activation(out=vT_sb[:, nt], in_=ps,
                                             func=AF.Identity, bias=0.0,
                                             scale=1.0)
                st[b]["q"], st[b]["k"], st[b]["vT"] = q_sb, k_sb, vT_sb

            def phase_attn(b):
                """scores->exp (fp8), row sums, AV, all DoubleRow fp8."""
                q_sb, k_sb, vT_sb = st[b]["q"], st[b]["k"], st[b]["vT"]
                eTs = [big.tile([128, NT, 512], F8, tag="eT", bufs=4,
                                name=f"eT_{b}_{ic}") for ic in range(IC)]
                for jt in range(NT):
                    pss = [psum.tile([128, 512], F32, tag="mm", bufs=6,
                                     name=f"sc_ps_{b}_{jt}_{ic}")
                           for ic in range(IC)]
                    for cp in range(CP):
                        for ic in range(IC):
                            nc.tensor.matmul(
                                pss[ic],
                                lhsT=k_sb[:, 2 * cp:2 * cp + 2,
                                          jt * 128:(jt + 1) * 128],
                                rhs=q_sb[:, 2 * cp:2 * cp + 2,
                                         ic * 512:(ic + 1) * 512],
                                start=(cp == 0), stop=(cp == CP - 1),
                                perf_mode=DR,
                            )
                    for ic in range(IC):
                        nc.scalar.activation(
                            out=eTs[ic][:, jt], in_=pss[ic], func=AF.Exp,
                            scale=scale, bias=ebias_sb,
                        )
                # r[i] = sum_j eT[j, i] over the fp8 eT the AV GEMM sees
                rs_pss = [psum.tile([16, 512], F32, tag="small", bufs=2,
                                    name=f"rs_ps_{b}_{ic}") for ic in range(IC)]
                for jp in range(JP):
                    for ic in range(IC):
                        nc.tensor.matmul(
                            rs_pss[ic], lhsT=ones8_sb,
                            rhs=eTs[ic][:, 2 * jp:2 * jp + 2, :],
                            start=(jp == 0), stop=(jp == JP - 1),
                            perf_mode=DR,
                        )
                rsums, rinvs = [], []
                for ic in range(IC):
                    lr_sb = small.tile([1, 512], F32, tag="lnr", bufs=2,
                                       name=f"lnr_{b}_{ic}")
                    nc.scalar.activation(out=lr_sb, in_=rs_pss[ic][0:1, :],
                                         func=AF.Ln)
                    rinv_sb = small.tile([1, 512], F32R, tag="rinv", bufs=2,
                                         name=f"rinv_{b}_{ic}")
                    nc.scalar.activation(out=rinv_sb, in_=lr_sb, func=AF.Exp,
                                         scale=-1.0)
                    rinvs.append(rinv_sb)
                    if use_bv:
                        rsum_sb = small.tile([1, 512], F32R, tag="rsum",
                                             bufs=2, name=f"rsum_{b}_{ic}")
                        nc.vector.tensor_copy(rsum_sb, rs_pss[ic][0:1, :])
                        rsums.append(rsum_sb)

                avns = [big.tile([128, CT, 512], F8, tag="avn", bufs=4,
                                 name=f"avn_{b}_{ic}") for ic in range(IC)]
                av_pss = []
                bc_pss = []
                for ct in range(CT):
                    pss = [psum.tile([128, 512], F32, tag="mm", bufs=6,
                                     name=f"av_ps_{b}_{ct}_{ic}")
                           for ic in range(IC)]
                    av_pss.append(pss)
                    for jp in range(JP):
                        for ic in range(IC):
                            nc.tensor.matmul(
                                pss[ic],
                                lhsT=vT_sb[:, 2 * jp:2 * jp + 2,
                                           ct * 128:(ct + 1) * 128],
                                rhs=eTs[ic][:, 2 * jp:2 * jp + 2, :],
                                start=(jp == 0),
                                stop=(jp == JP - 1 and not use_bv),
                                perf_mode=DR,
                            )
                    if use_bv:
                        for ic in range(IC):
                            nc.tensor.matmul(
                                pss[ic],
                                lhsT=bvr_sb[0:1, ct * 128:(ct + 1) * 128],
                                rhs=rsums[ic], start=False, stop=True,
                                skip_group_check=True,
                            )
                    if ct == 0:
                        # broadcast 1/r across partitions; placed after
                        # the first AV group so the PE does not idle on
                        # the ACT ln/exp chain above.
                        for ic in range(IC):
                            bc_ps = psum.tile([128, 512], F32, tag="mm",
                                              bufs=6, name=f"bc_ps_{b}_{ic}")
                            nc.tensor.matmul(bc_ps, lhsT=ones1_sb,
                                             rhs=rinvs[ic],
                                             start=True, stop=True)
                            bc_pss.append(bc_ps)
                rinvbs = []
                for ic in range(IC):
                    rinvb_sb = small.tile([128, 512], F32, tag="rinvb", bufs=4,
                                          name=f"rinvb_{b}_{ic}")
                    nc.vector.tensor_copy(rinvb_sb, bc_pss[ic])
                    rinvbs.append(rinvb_sb)
                for ct in range(CT):
                    for ic in range(IC):
                        nc.vector.tensor_mul(avns[ic][:, ct], av_pss[ct][ic],
                                             rinvbs[ic])
                st[b]["avn"] = avns

            def phase_proj(b):
                """y = Wo av + x (+bo), residual via identity matmul."""
                x_sb = x_sbs[b]
                avns = st[b]["avn"]
                for ot in range(CT):
                    pss = [psum.tile([128, 512], F32, tag="mm", bufs=6,
                                     name=f"pr_ps_{b}_{ot}_{ic}")
                           for ic in range(IC)]
                    for cp in range(CP):
                        for ic in range(IC):
                            nc.tensor.matmul(
                                pss[ic],
                                lhsT=wo_sb[:, 2 * cp:2 * cp + 2,
                                           ot * 128:(ot + 1) * 128],
                                rhs=avns[ic][:, 2 * cp:2 * cp + 2, :],
                                start=(cp == 0), stop=False,
                                perf_mode=DR, skip_group_check=True,
                            )
                    for ic in range(IC):
                        nc.tensor.matmul(
                            pss[ic], lhsT=id_sb,
                            rhs=x_sb[:, ot, ic * 512:(ic + 1) * 512],
                            start=False, stop=(not use_bo),
                            skip_group_check=True,
                        )
                    if use_bo:
                        for ic in range(IC):
                            nc.tensor.matmul(
                                pss[ic],
                                lhsT=bor_sb[0:1, ot * 128:(ot + 1) * 128],
                                rhs=ones512_sb, start=False, stop=True,
                                skip_group_check=True,
                            )
                    y_sb = big.tile([128, N], F32, tag="y", bufs=4,
                                    name=f"y_{b}_{ot}")
                    for ic in range(IC):
                        out = y_sb[:, ic * 512:(ic + 1) * 512]
                        if ic % 2 == 0:
                            nc.scalar.activation(out=out, in_=pss[ic],
                                                 func=AF.Identity, bias=0.0,
                                                 scale=1.0)
                        else:
                            nc.vector.tensor_copy(out, pss[ic])
                    yqs = ([nc.sync, nc.gpsimd, nc.sync, nc.gpsimd] if b == 0
                           else [nc.sync, nc.gpsimd, nc.scalar, nc.sync])
                    yqs[ot].dma_start(out=y_d[b, :, ot, :], in_=y_sb)

            # ---- interleaved build: issue order is scheduler priority ----
            phase_norm(0)
            # stage C: batch-1 x8 + batch-0 f32 x (~3 MB), gated on
            # batch-0's hn being fully written (so they can't steal DMA
            # bandwidth from stage A/B).
            with tc.tile_wait_until(0.0085, enable=False):
                nc.sync.dma_start(out=probes[:, 8:12],
                                  in_=st[0]["hn"][0:1, :, 1023:1024])
                nc.gpsimd.dma_start(out=probes[:, 12:16],
                                    in_=st[0]["hn"][0:1, :, 1022:1023])
            for ct in range(CT):
                dma([nc.sync, nc.gpsimd][ct % 2], 0.0090 + 0.0004 * ct,
                    x8_sbs[1][:, ct], x8_d[1, :, ct])
            dma(nc.sync, 0.0100, x_sbs[0][:, 0:2], x_d[0, :, 0:2])
            dma(nc.gpsimd, 0.0100, x_sbs[0][:, 2:4], x_d[0, :, 2:4])
            phase_qkv(0)
            # batch-1 groupnorm issued EARLY so its small DVE chain
            # outranks batch-0's eviction stream and hides under batch-0's
            # attention (its x8 gates it at runtime anyway).
            phase_norm(1)
            # stage D: batch-1 f32 x on the scalar queue, behind the ACT
            # stream's batch-0 qk evictions (fires ~mid-attention).
            dma(nc.scalar, 0.0200, x_sbs[1][:, 0:2], x_d[1, :, 0:2])
            dma(nc.scalar, 0.0210, x_sbs[1][:, 2:4], x_d[1, :, 2:4])
            phase_attn(0)
            phase_proj(0)
            phase_qkv(1)
            phase_attn(1)
            phase_proj(1)
    return nc


_CACHE = {}


def _get_nc(use_bq=False, use_bk=False, use_bv=False, use_bo=False):
    key = (use_bq, use_bk, use_bv, use_bo)
    if key not in _CACHE:
        _CACHE[key] = build_nc(*key)
    return _CACHE[key]


def prepare(x, norm_scale, norm_bias, wq, bq, wk, bk, wv, bv, wo, bo):
    """Host-side prep: returns (in_maps, flags)."""
    x = np.ascontiguousarray(np.asarray(x, dtype=np.float32))
    f32 = lambda a: np.asarray(a, dtype=np.float32)
    norm_scale, norm_bias = f32(norm_scale), f32(norm_bias)
    wq, wk, wv, wo = f32(wq), f32(wk), f32(wv), f32(wo)
    bq, bk, bv, bo = f32(bq), f32(bk), f32(bv), f32(bo)

    # [C, C] w  ->  wT[c, o] arranged [p, ct, o]
    def arr_w(w, dt):
        a = np.ascontiguousarray(w.T.reshape(CT, 128, C).transpose(1, 0, 2))
        return np.ascontiguousarray(a.astype(dt))

    # [C] vec (channel-tile major) -> [p, ct]
    def arr_c(v):
        return np.ascontiguousarray(v.reshape(CT, 128).T)

    S = np.zeros((128, GPT), np.float32)
    S[np.arange(128), np.arange(128) // GS] = 1.0
    pk1 = np.concatenate(
        [S, arr_c(norm_scale), arr_c(norm_bias), arr_c(bq), arr_c(bk)], axis=1)
    pk2 = np.concatenate(
        [np.ones(128, np.float32), np.ones(512, np.float32),
         bo.reshape(C), bv.reshape(C)]).reshape(1, -1)
    common = {
        "wqT": arr_w(wq, ml_dtypes.bfloat16),
        "wkT": arr_w(wk, ml_dtypes.bfloat16),
        "wvT": arr_w(wv, ml_dtypes.bfloat16),
        "woT8": arr_w(wo, ml_dtypes.float8_e4m3),
        "idm": np.ascontiguousarray(np.eye(128, dtype=np.float32)),
        "pk1": np.ascontiguousarray(pk1),
        "pk2": np.ascontiguousarray(pk2),
        "ST": np.ascontiguousarray(S.T),
    }

    # x: (B, C, H, W) -> per core [NB, p, ct, n]
    xf = x.reshape(B, C, N).reshape(B, CT, 128, N).transpose(0, 2, 1, 3)
    x8f = np.ascontiguousarray(xf.astype(ml_dtypes.bfloat16))
    in_maps = [
        {**common,
         "x": np.ascontiguousarray(xf[i * NB:(i + 1) * NB]),
         "x8": np.ascontiguousarray(x8f[i * NB:(i + 1) * NB])}
        for i in range(NCORES)
    ]
    flags = (bool(np.any(bq != 0.0)), bool(np.any(bk != 0.0)),
             bool(np.any(bv != 0.0)), bool(np.any(bo != 0.0)))
    return in_maps, flags


def assemble(results):
    y = np.empty((B, C, N), np.float32)
    for i in range(NCORES):
        yc = results[i]["y"]  # [NB, 128, CT, N]
        y[i * NB:(i + 1) * NB] = (
            yc.transpose(0, 2, 1, 3).reshape(NB, C, N))
    return y.reshape(B, C, H, W)


def kernel(x, norm_scale, norm_bias, wq, bq, wk, bk, wv, bv, wo, bo):
    in_maps, flags = prepare(x, norm_scale, norm_bias, wq, bq,
                             wk, bk, wv, bv, wo, bo)
    nc = _get_nc(*flags)
    res = run_bass_kernel_spmd(nc, in_maps, list(range(NCORES)))
    return assemble(res.results)


# revision 12
# speedup vs baseline: 1.2684x; 1.0631x over previous
"""Trainium2 Bass kernel for nn_AttnBlock (B=16, C=512, H=W=32).

Strategy
--------
Data-parallel over batch: 16 batch elements / 8 NeuronCores = 2 per core.
Per batch element (C=512 channels, N=1024 pixels), all on one core:

  1. GroupNorm(32 groups) in [c, n] layout, pipelined PER CHANNEL TILE
     over a bf16 copy of x (half the DMA bytes on the critical path;
     the f32 x streams later, used only for the residual).  Each
     128-channel tile's stats (bn_stats -> group aggregation via a tiny
     0/1-indicator PE matmul -> sqrt/reciprocal -> broadcast-back
     matmul) complete as soon as that tile's DMA lands; the
     hn = x*A + B apply (bf16 out) follows immediately.
  2. q = Wq hn, k = Wk hn, vT = (Wv hn)^T -- bf16 matmuls at full PE
     rate with half the weight DMA.  All three evict to fp8e4m3.
  3. Attention in fp8 DoubleRow matmuls (2 fp8 MACs per PE cell per
     cycle): eT[j,i] = exp(kq/sqrt(C) - 2) computed directly in [j, i]
     layout (the -2 bias keeps exp <= ~125 < 240, the TRN fp8e4 max;
     softmax normalization cancels it exactly).  Row sums via a
     DoubleRow ones-vector matmul (16-wide ones: dual-fp8 LDWEIGHTS
     needs a 16B-multiple pair step); 1/r via ACT ln/exp;
     av = (vT^T eT) * (1/r) evicted to fp8.
  4. proj: y = Wo av + x with Wo in fp8 DoubleRow and the residual x
     added INTO the proj PSUM by an identity-matrix f32r matmul over
     the f32 x, so the eviction is a pure copy.

Precision (sim, scale-relative absmax vs f32 reference): 1.08e-2 vs
the 2e-2 gate.  fp8 is applied only where the softmax structure damps
it; the residual path stays f32r-exact.

DMA queues (sync / gpsimd / scalar-early, ~72 GB/s each) are packed in
first-use order; evictions are balanced across ACT/DVE/GpSimd.  The
kernel graph is built once per process and reused.
"""
import contextlib
import os
import sys

for _p in ("/opt/trn_rl_repo",):
    if _p not in sys.path and os.path.isdir(_p):
        sys.path.append(_p)

import numpy as np
import ml_dtypes

import concourse.bass as bass
import concourse.tile as tile
from concourse import mybir
from concourse.bass_utils import run_bass_kernel_spmd
from concourse.vector_clock import ScopedClock

F32 = mybir.dt.float32
F32R = mybir.dt.float32r
BF16 = mybir.dt.bfloat16
F8 = mybir.dt.float8e4
AF = mybir.ActivationFunctionType
DR = mybir.MatmulPerfMode.DoubleRow

NCORES = 8
B, C, N = 16, 512, 1024
H = W = 32
NB = B // NCORES          # batch elements per core
CT = C // 128             # channel tiles of 128
NT = N // 128             # pixel tiles of 128
IC = N // 512             # query chunks of 512
CP = CT // 2              # channel-tile pairs (DoubleRow K=256)
JP = NT // 2              # pixel-tile pairs (DoubleRow K=256)
G, GS = 32, 16            # groups, channels per group
GPT = 128 // GS           # groups per 128-channel tile
EPS = 1e-6
EXP_BIAS = 2.0            # exp(s - 2): keeps eT <= ~125 < 240 (fp8e4 max)


class _TC(tile.TileContext):
    """TileContext with multi-wait instructions split for this walrus.

    The pinned walrus accepts at most one semaphore wait per instruction
    (two for EventSemaphore).  Tile's scheduler can attach several; the
    extras are moved onto no-op carriers committed immediately before on
    the same engine, which is semantically identical (engine streams are
    sequential).
    """

    def _commit_instruction(self, inst, lazy_reg_writes: bool = True):
        si = inst.sync_info
        cap = 2 if isinstance(inst, mybir.InstEventSemaphore) else 1
        if si is not None and si.on_wait and len(si.on_wait) > cap and \
                inst.engine != mybir.EngineType.Unassigned:
            waits = list(si.on_wait)
            inst.sync_info = mybir.SyncInfo(
                on_wait=waits[:cap], on_update=list(si.on_update or [])
            )
            for w in waits[cap:]:
                nop = mybir.InstNoOp(
                    name=self.nc.get_next_instruction_name(),
                    ins=[],
                    outs=[],
                    engine=inst.engine,
                    sync_info=mybir.SyncInfo(on_wait=[w], on_update=[]),
                    bass_nofuse=True,
                )
                super()._commit_instruction(nop, lazy_reg_writes=False)
        super()._commit_instruction(inst, lazy_reg_writes)

    def _drain_and_barrier(self, tick_clock, wait_clock):
        # Collect the final-tick waits on a probe drain, then distribute
        # them across all engines (one wait per carrier instruction).
        # Each engine then signals a star-barrier semaphore; gpsimd
        # collects all signals and clears the semaphores.  This replaces
        # Tile's two EVSEM-butterfly all-engine barriers (~10us).
        nc = self.nc
        drain_inst = nc.sync.drain()
        wait_clock.add_sem_waits(
            drain_inst.ins, ScopedClock({None: tick_clock.global_clock})
        )
        si = drain_inst.ins.sync_info
        waits = list(si.on_wait) if si and si.on_wait else []
        drain_inst.ins.sync_info = mybir.SyncInfo(
            on_wait=waits[:1], on_update=[]
        )
        engines = list(nc.engines.values())
        for i, w in enumerate(waits[1:]):
            eng = engines[i % len(engines)]
            nop = eng.nop(nofuse=True)
            nop.ins.sync_info = mybir.SyncInfo(on_wait=[w], on_update=[])
        star = nc.alloc_semaphore("tile_star_barrier")
        nsig = 0
        for eng in engines:
            if eng is not nc.gpsimd:
                eng.sem_inc(star, 1)
                nsig += 1
        nc.gpsimd.wait_ge(star, nsig)
        assert self.sems is not None
        popped = nc._tile_sem_poison_stack.pop()
        assert popped is self._sem_poison
        nc.clear_and_free_semaphores(
            list(self.sems.allocated().values()) + [star])


def build_nc(use_bq: bool, use_bk: bool, use_bv: bool, use_bo: bool):
    nc = bass.Bass()

    # Per-core DRAM I/O.  x8 is the bf16 copy (groupnorm path); x is the
    # f32 original, declared f32r so the PE identity-matmul residual add
    # can read it at full rate.
    x8_d = nc.declare_dram_parameter("x8", [NB, 128, CT, N], BF16, isOutput=False)
    x_d = nc.declare_dram_parameter("x", [NB, 128, CT, N], F32R, isOutput=False)
    y_d = nc.declare_dram_parameter("y", [NB, 128, CT, N], F32, isOutput=True)
    wq_d = nc.declare_dram_parameter("wqT", [128, CT, 512], BF16, isOutput=False)
    wk_d = nc.declare_dram_parameter("wkT", [128, CT, 512], BF16, isOutput=False)
    wv_d = nc.declare_dram_parameter("wvT", [128, CT, 512], BF16, isOutput=False)
    wo_d = nc.declare_dram_parameter("woT8", [128, CT, 512], F8, isOutput=False)
    id_d = nc.declare_dram_parameter("idm", [128, 128], F32R, isOutput=False)
    # pk1 packs [S | nsc | nbi | bqt | bkt] f32 columns.
    pk1_d = nc.declare_dram_parameter("pk1", [128, GPT + 4 * CT], F32,
                                      isOutput=False)
    # pk2 packs the f32r row constants [ones1(128) | ones512(512) |
    # bor(512) | bvr(512)].
    pk2_d = nc.declare_dram_parameter("pk2", [1, 128 + 3 * 512], F32R,
                                      isOutput=False)
    ST_d = nc.declare_dram_parameter("ST", [GPT, 128], F32, isOutput=False)

    scale = float(C) ** -0.5

    with _TC(nc) as tc:
        with (
            tc.tile_pool(name="consts", bufs=1) as consts,
            tc.tile_pool(name="big", bufs=1) as big,
            tc.tile_pool(name="small", bufs=2) as small,
            tc.tile_pool(name="psum", bufs=1, space="PSUM") as psum,
        ):
            # ---- constant + weight tiles ----
            pk1_sb = consts.tile([128, GPT + 4 * CT], F32, tag="pk1")
            ST_sb = consts.tile([GPT, 128], F32, tag="ST")
            pk2_sb = consts.tile([1, 128 + 3 * 512], F32R, tag="pk2")
            id_sb = consts.tile([128, 128], F32R, tag="idm")
            wq_sb = consts.tile([128, CT, 512], BF16, tag="wq")
            wk_sb = consts.tile([128, CT, 512], BF16, tag="wk")
            wv_sb = consts.tile([128, CT, 512], BF16, tag="wv")
            wo_sb = consts.tile([128, CT, 512], F8, tag="wo")
            x8_sbs = [big.tile([128, CT, N], BF16, tag="x8", bufs=2,
                               name=f"x8_{b}") for b in range(NB)]
            x_sbs = [big.tile([128, CT, N], F32R, tag="x", bufs=2,
                              name=f"x_{b}") for b in range(NB)]

            S_sb = pk1_sb[:, 0:GPT]
            nsc_sb = pk1_sb[:, GPT:GPT + CT]
            nbi_sb = pk1_sb[:, GPT + CT:GPT + 2 * CT]
            bqt_sb = pk1_sb[:, GPT + 2 * CT:GPT + 3 * CT]
            bkt_sb = pk1_sb[:, GPT + 3 * CT:GPT + 4 * CT]
            ones1_sb = pk2_sb[:, 0:128]
            ones512_sb = pk2_sb[:, 128:640]
            bor_sb = pk2_sb[:, 640:1152]
            bvr_sb = pk2_sb[:, 1152:1664]

            # ---- DMA schedule.  The 16 SDMA engines share ~360 GB/s and
            # run all triggered transfers CONCURRENTLY, so late transfers
            # must not be triggered early or they steal bandwidth from the
            # critical batch-0 x8 tiles.  Triggers are staged: stage A
            # fires immediately; later stages sit behind probe DMAs (or
            # compute) in the same engine stream, so they fire only once
            # the earlier stage's data has LANDED.  Floors (scheduler
            # hints) keep the modeled order consistent.
            def dma(eng, floor, out, in_):
                with tc.tile_wait_until(floor, enable=False):
                    eng.dma_start(out=out, in_=in_)

            probes = consts.tile([1, 16], BF16, tag="probe")
            # stage A: pk + batch-0 x8 + first half of wq (~1.3 MB)
            dma(nc.scalar, 0, pk1_sb, pk1_d[:, :])
            dma(nc.scalar, 0, ST_sb, ST_d[:, :])
            dma(nc.scalar, 0, pk2_sb, pk2_d[:, :])
            dma(nc.sync, 0, x8_sbs[0][:, 0], x8_d[0, :, 0])
            dma(nc.scalar, 0.0005, x8_sbs[0][:, 1], x8_d[0, :, 1])
            dma(nc.gpsimd, 0, x8_sbs[0][:, 2], x8_d[0, :, 2])
            dma(nc.sync, 0.0007, x8_sbs[0][:, 3], x8_d[0, :, 3])
            dma(nc.gpsimd, 0.0007, wq_sb[:, 0:2], wq_d[:, 0:2])
            # stage B, gated on all of batch-0 x8 having landed: rest of
            # the weights (~1.5 MB).
            with tc.tile_wait_until(0.0045, enable=False):
                nc.sync.dma_start(out=probes[:, 0:4],
                                  in_=x8_sbs[0][0:1, :, 1023:1024])
                nc.gpsimd.dma_start(out=probes[:, 4:8],
                                    in_=x8_sbs[0][0:1, :, 1022:1023])
            dma(nc.sync, 0.0050, wq_sb[:, 2:4], wq_d[:, 2:4])
            dma(nc.sync, 0.0052, wk_sb, wk_d[:, :, :])
            dma(nc.gpsimd, 0.0050, wv_sb, wv_d[:, :, :])
            dma(nc.gpsimd, 0.0054, wo_sb, wo_d[:, :, :])
            dma(nc.gpsimd, 0.0056, id_sb, id_d[:, :])
            # stage C (batch-1 x8 + batch-0 f32 x) is issued after
            # phase_qkv(0) below, behind a probe on batch-0's hn.

            # bn_stats floors: stage-A arrival estimates.
            arrive_ms = {
                0: {(ct, h): [0.0045, 0.0052, 0.0058, 0.0062][ct]
                    for ct in range(CT) for h in range(2)},
                1: {(ct, h): 0.0130 + 0.0010 * ct for ct in range(CT)
                    for h in range(2)},
            }

            eps_sb = consts.tile([GPT, 1], F32, tag="eps")
            nc.vector.memset(eps_sb, EPS)
            ebias_sb = consts.tile([128, 1], F32, tag="ebias")
            nc.vector.memset(ebias_sb, -EXP_BIAS)
            # ones for the DoubleRow row-sum; 16 columns because dual-fp8
            # LDWEIGHTS needs the pair-dim step to be a multiple of 16B.
            ones8_sb = consts.tile([128, 2, 16], F8, tag="ones8")
            nc.vector.memset(ones8_sb, 1.0)
            # Warm the ACT tables used later (first use of a function
            # pays the table load) while DMAs stream.
            for wf, wname in ((AF.Sqrt, "sqw"), (AF.Exp, "exw"),
                              (AF.Ln, "lnw"), (AF.Identity, "idw")):
                wt = consts.tile([GPT, 1], F32, tag=wname)
                nc.scalar.activation(out=wt, in_=eps_sb, func=wf,
                                     bias=0.0, scale=1.0)

            # Per-batch state carried across the phase interleave below.
            st = [dict() for _ in range(NB)]

            def phase_norm(b):
                """GroupNorm: per-tile bn_stats as DMAs land, ONE batched
                join (x8 tiles all land within ~2us of each other, so a
                single join has fewer DVE ops and fewer in-order stalls
                than per-tile chains)."""
                x8_sb = x8_sbs[b]
                hn_sb = big.tile([128, CT, N], BF16, tag="hn", bufs=2,
                                 name=f"hn{b}")
                gstats = small.tile([GPT, CT, 2], F32, tag="gstats",
                                    name=f"gstats{b}")
                for ct in range(CT):
                    stats = small.tile([128, 2, 6], F32, tag=f"bnst{ct}",
                                       name=f"bnst_{b}_{ct}")
                    ts = small.tile([128, 2], F32, tag=f"ts{ct}",
                                    name=f"ts_{b}_{ct}")
                    mv = small.tile([128, 2], F32, tag=f"mv{ct}",
                                    name=f"mv_{b}_{ct}")
                    for h in range(2):
                        with tc.tile_wait_until(arrive_ms[b][(ct, h)],
                                                enable=False):
                            nc.vector.bn_stats(
                                out=stats[:, h],
                                in_=x8_sb[:, ct, h * 512:(h + 1) * 512],
                            )
                    nc.vector.bn_aggr(out=mv, in_=stats)
                    nc.vector.tensor_copy(ts[:, 0:1], mv[:, 0:1])
                    nc.vector.tensor_mul(ts[:, 1:2], mv[:, 0:1], mv[:, 0:1])
                    nc.vector.tensor_add(ts[:, 1:2], ts[:, 1:2], mv[:, 1:2])
                    ps = psum.tile([GPT, 2], F32, tag="mm", bufs=6,
                                   name=f"stat_ps_{b}_{ct}")
                    nc.tensor.matmul(ps, lhsT=S_sb, rhs=ts,
                                     start=True, stop=True)
                    nc.vector.tensor_copy(gstats[:, ct], ps)
                # join: group mean / rstd for all tiles at once
                gm = small.tile([GPT, CT, 2], F32, tag="gm", name=f"gm{b}")
                nc.vector.tensor_scalar_mul(gm, gstats, 1.0 / GS)
                tmp8 = small.tile([GPT, CT], F32, tag="tmp8", name=f"tmp8{b}")
                nc.vector.tensor_mul(tmp8, gm[:, :, 0], gm[:, :, 0])
                nc.vector.tensor_sub(gm[:, :, 1], gm[:, :, 1], tmp8)
                nc.scalar.activation(out=gm[:, :, 1], in_=gm[:, :, 1],
                                     func=AF.Sqrt, bias=eps_sb, scale=1.0)
                nc.vector.reciprocal(gm[:, :, 1], gm[:, :, 1])
                AB = small.tile([128, CT, 2], F32, tag="AB", name=f"AB{b}")
                for ct in range(CT):
                    ps2 = psum.tile([128, 2], F32, tag="mm", bufs=6,
                                    name=f"ab_ps_{b}_{ct}")
                    nc.tensor.matmul(ps2, lhsT=ST_sb, rhs=gm[:, ct],
                                     start=True, stop=True)
                    nc.vector.tensor_copy(AB[:, ct], ps2)
                A_sb = small.tile([128, CT], F32, tag="A", name=f"A{b}")
                B_sb = small.tile([128, CT], F32, tag="B", name=f"B{b}")
                nc.vector.tensor_mul(A_sb, AB[:, :, 1], nsc_sb)
                nc.vector.tensor_mul(B_sb, AB[:, :, 0], A_sb)
                nc.vector.tensor_sub(B_sb, nbi_sb, B_sb)
                for ct in range(CT):
                    # hn = x*A + B (bf16 out); two tiles on GpSimd (slow
                    # but idle), two on DVE's fast path.
                    eng = nc.gpsimd if ct < 2 else nc.vector
                    eng.tensor_scalar(
                        out=hn_sb[:, ct], in0=x8_sb[:, ct],
                        scalar1=A_sb[:, ct:ct + 1],
                        scalar2=B_sb[:, ct:ct + 1],
                        op0=mybir.AluOpType.mult, op1=mybir.AluOpType.add,
                    )
                st[b]["hn"] = hn_sb

            def phase_qkv(b):
                """q, k (fp8 out) in [c, n]; vT (fp8 out) in [n, c]."""
                hn_sb = st[b]["hn"]
                q_sb = big.tile([128, CT, N], F8, tag="q", bufs=2,
                                name=f"q{b}")
                k_sb = big.tile([128, CT, N], F8, tag="k", bufs=2,
                                name=f"k{b}")
                evict_i = 0
                for wname, w_sb, dst, bias_sb, use_b in (
                        ("q", wq_sb, q_sb, bqt_sb, use_bq),
                        ("k", wk_sb, k_sb, bkt_sb, use_bk)):
                    for ot in range(CT):
                        pss = [psum.tile([128, 512], F32, tag="mm", bufs=6,
                                         name=f"{wname}_ps_{b}_{ot}_{ic}")
                               for ic in range(IC)]
                        for ct in range(CT):
                            for ic in range(IC):
                                nc.tensor.matmul(
                                    pss[ic],
                                    lhsT=w_sb[:, ct, ot * 128:(ot + 1) * 128],
                                    rhs=hn_sb[:, ct, ic * 512:(ic + 1) * 512],
                                    start=(ct == 0), stop=(ct == CT - 1),
                                )
                        for ic in range(IC):
                            out = dst[:, ot, ic * 512:(ic + 1) * 512]
                            if use_b:
                                if evict_i % 2 == 0:
                                    nc.vector.tensor_scalar_add(
                                        out, pss[ic], bias_sb[:, ot:ot + 1])
                                else:
                                    nc.scalar.activation(
                                        out=out, in_=pss[ic], func=AF.Identity,
                                        bias=bias_sb[:, ot:ot + 1], scale=1.0)
                            else:
                                if evict_i % 2 == 0:
                                    nc.vector.tensor_copy(out, pss[ic])
                                else:
                                    nc.scalar.activation(
                                        out=out, in_=pss[ic], func=AF.Identity,
                                        bias=0.0, scale=1.0)
                            evict_i += 1
                vT_sb = big.tile([128, NT, 512], F8, tag="vT", bufs=2,
                                 name=f"vT{b}")
                for nt in range(NT):
                    ps = psum.tile([128, 512], F32, tag="mm", bufs=6,
                                   name=f"v_ps_{b}_{nt}")
                    for ct in range(CT):
                        nc.tensor.matmul(
                            ps,
                            lhsT=hn_sb[:, ct, nt * 128:(nt + 1) * 128],
                            rhs=wv_sb[:, ct, :],
                            start=(ct == 0), stop=(ct == CT - 1),
                        )
                    if nt % 2 == 0:
                        nc.vector.tensor_copy(vT_sb[:, nt], ps)
                    else:
                        nc.scalar.activation(out=vT_sb[:, nt], in_=ps,
                                             func=AF.Identity, bias=0.0,
                                             scale=1.0)
                st[b]["q"], st[b]["k"], st[b]["vT"] = q_sb, k_sb, vT_sb

            def phase_attn(b):
                """scores->exp (fp8), row sums, AV, all DoubleRow fp8."""
                q_sb, k_sb, vT_sb = st[b]["q"], st[b]["k"], st[b]["vT"]
                eTs = [big.tile([128, NT, 512], F8, tag="eT", bufs=4,
                                name=f"eT_{b}_{ic}") for ic in range(IC)]
                for jt in range(NT):
                    pss = [psum.tile([128, 512], F32, tag="mm", bufs=6,
                                     name=f"sc_ps_{b}_{jt}_{ic}")
                           for ic in range(IC)]
                    for cp in range(CP):
                        for ic in range(IC):
                            nc.tensor.matmul(
                                pss[ic],
                                lhsT=k_sb[:, 2 * cp:2 * cp + 2,
                                          jt * 128:(jt + 1) * 128],
                                rhs=q_sb[:, 2 * cp:2 * cp + 2,
                                         ic * 512:(ic + 1) * 512],
                                start=(cp == 0), stop=(cp == CP - 1),
                                perf_mode=DR,
                            )
                    for ic in range(IC):
                        nc.scalar.activation(
                            out=eTs[ic][:, jt], in_=pss[ic], func=AF.Exp,
                            scale=scale, bias=ebias_sb,
                        )
                # r[i] = sum_j eT[j, i] over the fp8 eT the AV GEMM sees
                rs_pss = [psum.tile([16, 512], F32, tag="small", bufs=2,
                                    name=f"rs_ps_{b}_{ic}") for ic in range(IC)]
                for jp in range(JP):
                    for ic in range(IC):
                        nc.tensor.matmul(
                            rs_pss[ic], lhsT=ones8_sb,
                            rhs=eTs[ic][:, 2 * jp:2 * jp + 2, :],
                            start=(jp == 0), stop=(jp == JP - 1),
                            perf_mode=DR,
                        )
                rsums, rinvs = [], []
                for ic in range(IC):
                    lr_sb = small.tile([1, 512], F32, tag="lnr", bufs=2,
                                       name=f"lnr_{b}_{ic}")
                    nc.scalar.activation(out=lr_sb, in_=rs_pss[ic][0:1, :],
                                         func=AF.Ln)
                    rinv_sb = small.tile([1, 512], F32R, tag="rinv", bufs=2,
                                         name=f"rinv_{b}_{ic}")
                    nc.scalar.activation(out=rinv_sb, in_=lr_sb, func=AF.Exp,
                                         scale=-1.0)
                    rinvs.append(rinv_sb)
                    if use_bv:
                        rsum_sb = small.tile([1, 512], F32R, tag="rsum",
                                             bufs=2, name=f"rsum_{b}_{ic}")
                        nc.vector.tensor_copy(rsum_sb, rs_pss[ic][0:1, :])
                        rsums.append(rsum_sb)

                avns = [big.tile([128, CT, 512], F8, tag="avn", bufs=4,
                                 name=f"avn_{b}_{ic}") for ic in range(IC)]
                av_pss = []
                bc_pss = []
                for ct in range(CT):
                    pss = [psum.tile([128, 512], F32, tag="mm", bufs=6,
                                     name=f"av_ps_{b}_{ct}_{ic}")
                           for ic in range(IC)]
                    av_pss.append(pss)
                    for jp in range(JP):
                        for ic in range(IC):
                            nc.tensor.matmul(
                                pss[ic],
                                lhsT=vT_sb[:, 2 * jp:2 * jp + 2,
                                           ct * 128:(ct + 1) * 128],
                                rhs=eTs[ic][:, 2 * jp:2 * jp + 2, :],
                                start=(jp == 0),
                                stop=(jp == JP - 1 and not use_bv),
                                perf_mode=DR,
                            )
                    if use_bv:
                        for ic in range(IC):
                            nc.tensor.matmul(
                                pss[ic],
                                lhsT=bvr_sb[0:1, ct * 128:(ct + 1) * 128],
                                rhs=rsums[ic], start=False, stop=True,
                                skip_group_check=True,
                            )
                    if ct == 0:
                        # broadcast 1/r across partitions; placed after
                        # the first AV group so the PE does not idle on
                        # the ACT ln/exp chain above.
                        for ic in range(IC):
                            bc_ps = psum.tile([128, 512], F32, tag="mm",
                                              bufs=6, name=f"bc_ps_{b}_{ic}")
                            nc.tensor.matmul(bc_ps, lhsT=ones1_sb,
                                             rhs=rinvs[ic],
                                             start=True, stop=True)
                            bc_pss.append(bc_ps)
                rinvbs = []
                for ic in range(IC):
                    rinvb_sb = small.tile([128, 512], F32, tag="rinvb", bufs=4,
                                          name=f"rinvb_{b}_{ic}")
                    nc.vector.tensor_copy(rinvb_sb, bc_pss[ic])
                    rinvbs.append(rinvb_sb)
                for ct in range(CT):
                    for ic in range(IC):
                        nc.vector.tensor_mul(avns[ic][:, ct], av_pss[ct][ic],
                                             rinvbs[ic])
                st[b]["avn"] = avns

            def phase_proj(b):
                """y = Wo av + x (+bo), residual via identity matmul."""
                x_sb = x_sbs[b]
                avns = st[b]["avn"]
                for ot in range(CT):
                    pss = [psum.tile([128, 512], F32, tag="mm", bufs=6,
                                     name=f"pr_ps_{b}_{ot}_{ic}")
                           for ic in range(IC)]
                    for cp in range(CP):
                        for ic in range(IC):
                            nc.tensor.matmul(
                                pss[ic],
                                lhsT=wo_sb[:, 2 * cp:2 * cp + 2,
                                           ot * 128:(ot + 1) * 128],
                                rhs=avns[ic][:, 2 * cp:2 * cp + 2, :],
                                start=(cp == 0), stop=False,
                                perf_mode=DR, skip_group_check=True,
                            )
                    for ic in range(IC):
                        nc.tensor.matmul(
                            pss[ic], lhsT=id_sb,
                            rhs=x_sb[:, ot, ic * 512:(ic + 1) * 512],
                            start=False, stop=(not use_bo),
                            skip_group_check=True,
                        )
                    if use_bo:
                        for ic in range(IC):
                            nc.tensor.matmul(
                                pss[ic],
                                lhsT=bor_sb[0:1, ot * 128:(ot + 1) * 128],
                                rhs=ones512_sb, start=False, stop=True,
                                skip_group_check=True,
                            )
                    y_sb = big.tile([128, N], F32, tag="y", bufs=4,
                                    name=f"y_{b}_{ot}")
                    for ic in range(IC):
                        out = y_sb[:, ic * 512:(ic + 1) * 512]
                        if ic % 2 == 0:
                            nc.scalar.activation(out=out, in_=pss[ic],
                                                 func=AF.Identity, bias=0.0,
                                                 scale=1.0)
                        else:
                            nc.vector.tensor_copy(out, pss[ic])
                    yqs = ([nc.sync, nc.gpsimd, nc.sync, nc.gpsimd] if b == 0
                           else [nc.sync, nc.gpsimd, nc.scalar, nc.sync])
                    yqs[ot].dma_start(out=y_d[b, :, ot, :], in_=y_sb)

            # ---- interleaved build: issue order is scheduler priority ----
            phase_norm(0)
            # stage C: batch-1 x8 + batch-0 f32 x (~3 MB), gated on the
            # second half of wq having landed (so C can't steal DMA
            # bandwidth from batch-0's x8).
            with tc.tile_wait_until(0.0072, enable=False):
                nc.sync.dma_start(out=probes[:, 8:9],
                                  in_=wq_sb[0:1, 3, 511:512])
                nc.gpsimd.dma_start(out=probes[:, 12:13],
                                    in_=wq_sb[0:1, 3, 510:511])
            for ct in range(CT):
                dma([nc.sync, nc.gpsimd][ct % 2], 0.0075 + 0.0003 * ct,
                    x8_sbs[1][:, ct], x8_d[1, :, ct])
            dma(nc.sync, 0.0082, x_sbs[0][:, 0:2], x_d[0, :, 0:2])
            dma(nc.gpsimd, 0.0082, x_sbs[0][:, 2:4], x_d[0, :, 2:4])
            phase_qkv(0)
            # batch-1 groupnorm issued EARLY so its small DVE chain
            # outranks batch-0's eviction stream and hides under batch-0's
            # attention (its x8 gates it at runtime anyway).
            phase_norm(1)
            # stage D: batch-1 f32 x on the scalar queue, behind the ACT
            # stream's batch-0 qk evictions (fires ~mid-attention).
            dma(nc.scalar, 0.0200, x_sbs[1][:, 0:2], x_d[1, :, 0:2])
            dma(nc.scalar, 0.0210, x_sbs[1][:, 2:4], x_d[1, :, 2:4])
            phase_attn(0)
            phase_proj(0)
            phase_qkv(1)
            phase_attn(1)
            phase_proj(1)
    return nc


_CACHE = {}


def _get_nc(use_bq=False, use_bk=False, use_bv=False, use_bo=False):
    key = (use_bq, use_bk, use_bv, use_bo)
    if key not in _CACHE:
        _CACHE[key] = build_nc(*key)
    return _CACHE[key]


def prepare(x, norm_scale, norm_bias, wq, bq, wk, bk, wv, bv, wo, bo):
    """Host-side prep: returns (in_maps, flags)."""
    x = np.ascontiguousarray(np.asarray(x, dtype=np.float32))
    f32 = lambda a: np.asarray(a, dtype=np.float32)
    norm_scale, norm_bias = f32(norm_scale), f32(norm_bias)
    wq, wk, wv, wo = f32(wq), f32(wk), f32(wv), f32(wo)
    bq, bk, bv, bo = f32(bq), f32(bk), f32(bv), f32(bo)

    # [C, C] w  ->  wT[c, o] arranged [p, ct, o]
    def arr_w(w, dt):
        a = np.ascontiguousarray(w.T.reshape(CT, 128, C).transpose(1, 0, 2))
        return np.ascontiguousarray(a.astype(dt))

    # [C] vec (channel-tile major) -> [p, ct]
    def arr_c(v):
        return np.ascontiguousarray(v.reshape(CT, 128).T)

    S = np.zeros((128, GPT), np.float32)
    S[np.arange(128), np.arange(128) // GS] = 1.0
    pk1 = np.concatenate(
        [S, arr_c(norm_scale), arr_c(norm_bias), arr_c(bq), arr_c(bk)], axis=1)
    pk2 = np.concatenate(
        [np.ones(128, np.float32), np.ones(512, np.float32),
         bo.reshape(C), bv.reshape(C)]).reshape(1, -1)
    common = {
        "wqT": arr_w(wq, ml_dtypes.bfloat16),
        "wkT": arr_w(wk, ml_dtypes.bfloat16),
        "wvT": arr_w(wv, ml_dtypes.bfloat16),
        "woT8": arr_w(wo, ml_dtypes.float8_e4m3),
        "idm": np.ascontiguousarray(np.eye(128, dtype=np.float32)),
        "pk1": np.ascontiguousarray(pk1),
        "pk2": np.ascontiguousarray(pk2),
        "ST": np.ascontiguousarray(S.T),
    }

    # x: (B, C, H, W) -> per core [NB, p, ct, n]
    xf = x.reshape(B, C, N).reshape(B, CT, 128, N).transpose(0, 2, 1, 3)
    x8f = np.ascontiguousarray(xf.astype(ml_dtypes.bfloat16))
    in_maps = [
        {**common,
         "x": np.ascontiguousarray(xf[i * NB:(i + 1) * NB]),
         "x8": np.ascontiguousarray(x8f[i * NB:(i + 1) * NB])}
        for i in range(NCORES)
    ]
    flags = (bool(np.any(bq != 0.0)), bool(np.any(bk != 0.0)),
             bool(np.any(bv != 0.0)), bool(np.any(bo != 0.0)))
    return in_maps, flags


def assemble(results):
    y = np.empty((B, C, N), np.float32)
    for i in range(NCORES):
        yc = results[i]["y"]  # [NB, 128, CT, N]
        y[i * NB:(i + 1) * NB] = (
            yc.transpose(0, 2, 1, 3).reshape(NB, C, N))
    return y.reshape(B, C, H, W)


def kernel(x, norm_scale, norm_bias, wq, bq, wk, bk, wv, bv, wo, bo):
    in_maps, flags = prepare(x, norm_scale, norm_bias, wq, bq,
                             wk, bk, wv, bv, wo, bo)
    nc = _get_nc(*flags)
    res = run_bass_kernel_spmd(nc, in_maps, list(range(NCORES)))
    return assemble(res.results)


# revision 13
# speedup vs baseline: 1.3512x; 1.0652x over previous
"""Trainium2 Bass kernel for nn_AttnBlock (B=16, C=512, H=W=32).

Strategy
--------
Data-parallel over batch: 16 batch elements / 8 NeuronCores = 2 per core.
Per batch element (C=512 channels, N=1024 pixels), all on one core:

  1. GroupNorm(32 groups) in [c, n] layout, pipelined PER CHANNEL TILE
     over a bf16 copy of x (half the DMA bytes on the critical path;
     the f32 x streams later, used only for the residual).  Each
     128-channel tile's stats (bn_stats -> group aggregation via a tiny
     0/1-indicator PE matmul -> sqrt/reciprocal -> broadcast-back
     matmul) complete as soon as that tile's DMA lands; the
     hn = x*A + B apply (bf16 out) follows immediately.
  2. q = Wq hn, k = Wk hn, vT = (Wv hn)^T -- bf16 matmuls at full PE
     rate with half the weight DMA.  All three evict to fp8e4m3.
  3. Attention in fp8 DoubleRow matmuls (2 fp8 MACs per PE cell per
     cycle): eT[j,i] = exp(kq/sqrt(C) - 2) computed directly in [j, i]
     layout (the -2 bias keeps exp <= ~125 < 240, the TRN fp8e4 max;
     softmax normalization cancels it exactly).  Row sums via a
     DoubleRow ones-vector matmul (16-wide ones: dual-fp8 LDWEIGHTS
     needs a 16B-multiple pair step); 1/r via ACT ln/exp;
     av = (vT^T eT) * (1/r) evicted to fp8.
  4. proj: y = Wo av + x with Wo in fp8 DoubleRow and the residual x
     added INTO the proj PSUM by an identity-matrix f32r matmul over
     the f32 x, so the eviction is a pure copy.

Precision (sim, scale-relative absmax vs f32 reference): 1.08e-2 vs
the 2e-2 gate.  fp8 is applied only where the softmax structure damps
it; the residual path stays f32r-exact.

DMA queues (sync / gpsimd / scalar-early, ~72 GB/s each) are packed in
first-use order; evictions are balanced across ACT/DVE/GpSimd.  The
kernel graph is built once per process and reused.
"""
import contextlib
import os
import sys

for _p in ("/opt/trn_rl_repo",):
    if _p not in sys.path and os.path.isdir(_p):
        sys.path.append(_p)

import numpy as np
import ml_dtypes

import concourse.bass as bass
import concourse.tile as tile
from concourse import mybir
from concourse.bass_utils import run_bass_kernel_spmd
from concourse.vector_clock import ScopedClock

F32 = mybir.dt.float32
F32R = mybir.dt.float32r
BF16 = mybir.dt.bfloat16
F8 = mybir.dt.float8e4
AF = mybir.ActivationFunctionType
DR = mybir.MatmulPerfMode.DoubleRow

NCORES = 8
B, C, N = 16, 512, 1024
H = W = 32
NB = B // NCORES          # batch elements per core
CT = C // 128             # channel tiles of 128
NT = N // 128             # pixel tiles of 128
IC = N // 512             # query chunks of 512
CP = CT // 2              # channel-tile pairs (DoubleRow K=256)
JP = NT // 2              # pixel-tile pairs (DoubleRow K=256)
G, GS = 32, 16            # groups, channels per group
GPT = 128 // GS           # groups per 128-channel tile
EPS = 1e-6
EXP_BIAS = 2.0            # exp(s - 2): keeps eT <= ~125 < 240 (fp8e4 max)


class _TC(tile.TileContext):
    """TileContext with multi-wait instructions split for this walrus.

    The pinned walrus accepts at most one semaphore wait per instruction
    (two for EventSemaphore).  Tile's scheduler can attach several; the
    extras are moved onto no-op carriers committed immediately before on
    the same engine, which is semantically identical (engine streams are
    sequential).
    """

    def _commit_instruction(self, inst, lazy_reg_writes: bool = True):
        si = inst.sync_info
        cap = 2 if isinstance(inst, mybir.InstEventSemaphore) else 1
        if si is not None and si.on_wait and len(si.on_wait) > cap and \
                inst.engine != mybir.EngineType.Unassigned:
            waits = list(si.on_wait)
            inst.sync_info = mybir.SyncInfo(
                on_wait=waits[:cap], on_update=list(si.on_update or [])
            )
            for w in waits[cap:]:
                nop = mybir.InstNoOp(
                    name=self.nc.get_next_instruction_name(),
                    ins=[],
                    outs=[],
                    engine=inst.engine,
                    sync_info=mybir.SyncInfo(on_wait=[w], on_update=[]),
                    bass_nofuse=True,
                )
                super()._commit_instruction(nop, lazy_reg_writes=False)
        super()._commit_instruction(inst, lazy_reg_writes)

    def _drain_and_barrier(self, tick_clock, wait_clock):
        # Collect the final-tick waits on a probe drain, then distribute
        # them across all engines (one wait per carrier instruction).
        # Each engine then signals a star-barrier semaphore; gpsimd
        # collects all signals and clears the semaphores.  This replaces
        # Tile's two EVSEM-butterfly all-engine barriers (~10us).
        nc = self.nc
        drain_inst = nc.sync.drain()
        wait_clock.add_sem_waits(
            drain_inst.ins, ScopedClock({None: tick_clock.global_clock})
        )
        si = drain_inst.ins.sync_info
        waits = list(si.on_wait) if si and si.on_wait else []
        drain_inst.ins.sync_info = mybir.SyncInfo(
            on_wait=waits[:1], on_update=[]
        )
        engines = list(nc.engines.values())
        for i, w in enumerate(waits[1:]):
            eng = engines[i % len(engines)]
            nop = eng.nop(nofuse=True)
            nop.ins.sync_info = mybir.SyncInfo(on_wait=[w], on_update=[])
        star = nc.alloc_semaphore("tile_star_barrier")
        nsig = 0
        for eng in engines:
            if eng is not nc.gpsimd:
                eng.sem_inc(star, 1)
                nsig += 1
        nc.gpsimd.wait_ge(star, nsig)
        assert self.sems is not None
        popped = nc._tile_sem_poison_stack.pop()
        assert popped is self._sem_poison
        nc.clear_and_free_semaphores(
            list(self.sems.allocated().values()) + [star])


def build_nc(use_bq: bool, use_bk: bool, use_bv: bool, use_bo: bool):
    nc = bass.Bass()

    # Per-core DRAM I/O.  x8 is the bf16 copy (groupnorm path); x is the
    # f32 original, declared f32r so the PE identity-matmul residual add
    # can read it at full rate.
    x8_d = nc.declare_dram_parameter("x8", [NB, 128, CT, N], BF16, isOutput=False)
    x_d = nc.declare_dram_parameter("x", [NB, 128, CT, N], F32R, isOutput=False)
    y_d = nc.declare_dram_parameter("y", [NB, 128, CT, N], F32, isOutput=True)
    wq_d = nc.declare_dram_parameter("wqT", [128, CT, 512], BF16, isOutput=False)
    wk_d = nc.declare_dram_parameter("wkT", [128, CT, 512], BF16, isOutput=False)
    wv_d = nc.declare_dram_parameter("wvT", [128, CT, 512], BF16, isOutput=False)
    wo_d = nc.declare_dram_parameter("woT8", [128, CT, 512], F8, isOutput=False)
    id_d = nc.declare_dram_parameter("idm", [128, 128], F32R, isOutput=False)
    # pk1 packs [S | nsc | nbi | bqt | bkt] f32 columns.
    pk1_d = nc.declare_dram_parameter("pk1", [128, GPT + 4 * CT], F32,
                                      isOutput=False)
    # pk2 packs the f32r row constants [ones1(128) | ones512(512) |
    # bor(512) | bvr(512)].
    pk2_d = nc.declare_dram_parameter("pk2", [1, 128 + 3 * 512], F32R,
                                      isOutput=False)
    ST_d = nc.declare_dram_parameter("ST", [GPT, 128], F32, isOutput=False)

    scale = float(C) ** -0.5

    with _TC(nc) as tc:
        with (
            tc.tile_pool(name="consts", bufs=1) as consts,
            tc.tile_pool(name="big", bufs=1) as big,
            tc.tile_pool(name="small", bufs=2) as small,
            tc.tile_pool(name="psum", bufs=1, space="PSUM") as psum,
        ):
            # ---- constant + weight tiles ----
            pk1_sb = consts.tile([128, GPT + 4 * CT], F32, tag="pk1")
            ST_sb = consts.tile([GPT, 128], F32, tag="ST")
            pk2_sb = consts.tile([1, 128 + 3 * 512], F32R, tag="pk2")
            id_sb = consts.tile([128, 128], F32R, tag="idm")
            wq_sb = consts.tile([128, CT, 512], BF16, tag="wq")
            wk_sb = consts.tile([128, CT, 512], BF16, tag="wk")
            wv_sb = consts.tile([128, CT, 512], BF16, tag="wv")
            wo_sb = consts.tile([128, CT, 512], F8, tag="wo")
            x8_sbs = [big.tile([128, CT, N], BF16, tag="x8", bufs=2,
                               name=f"x8_{b}") for b in range(NB)]
            x_sbs = [big.tile([128, CT, N], F32R, tag="x", bufs=2,
                              name=f"x_{b}") for b in range(NB)]

            S_sb = pk1_sb[:, 0:GPT]
            nsc_sb = pk1_sb[:, GPT:GPT + CT]
            nbi_sb = pk1_sb[:, GPT + CT:GPT + 2 * CT]
            bqt_sb = pk1_sb[:, GPT + 2 * CT:GPT + 3 * CT]
            bkt_sb = pk1_sb[:, GPT + 3 * CT:GPT + 4 * CT]
            ones1_sb = pk2_sb[:, 0:128]
            ones512_sb = pk2_sb[:, 128:640]
            bor_sb = pk2_sb[:, 640:1152]
            bvr_sb = pk2_sb[:, 1152:1664]

            # ---- DMA schedule.  The 16 SDMA engines share ~360 GB/s and
            # run all triggered transfers CONCURRENTLY, so late transfers
            # must not be triggered early or they steal bandwidth from the
            # critical batch-0 x8 tiles.  Triggers are staged: stage A
            # fires immediately; later stages sit behind probe DMAs (or
            # compute) in the same engine stream, so they fire only once
            # the earlier stage's data has LANDED.  Floors (scheduler
            # hints) keep the modeled order consistent.
            def dma(eng, floor, out, in_):
                with tc.tile_wait_until(floor, enable=True):
                    eng.dma_start(out=out, in_=in_)

            probes = consts.tile([1, 16], BF16, tag="probe")
            # stage A: pk + batch-0 x8 + first half of wq (~1.3 MB)
            dma(nc.scalar, 0, pk1_sb, pk1_d[:, :])
            dma(nc.scalar, 0, ST_sb, ST_d[:, :])
            dma(nc.scalar, 0, pk2_sb, pk2_d[:, :])
            dma(nc.sync, 0, x8_sbs[0][:, 0], x8_d[0, :, 0])
            dma(nc.scalar, 0.0005, x8_sbs[0][:, 1], x8_d[0, :, 1])
            dma(nc.gpsimd, 0, x8_sbs[0][:, 2], x8_d[0, :, 2])
            dma(nc.sync, 0.0007, x8_sbs[0][:, 3], x8_d[0, :, 3])
            dma(nc.gpsimd, 0.0007, wq_sb[:, 0:2], wq_d[:, 0:2])
            # stage B, gated on all of batch-0 x8 having landed: rest of
            # the weights (~1.5 MB).
            with tc.tile_wait_until(0.0045, enable=True):
                nc.sync.dma_start(out=probes[:, 0:4],
                                  in_=x8_sbs[0][0:1, :, 1023:1024])
                nc.gpsimd.dma_start(out=probes[:, 4:8],
                                    in_=x8_sbs[0][0:1, :, 1022:1023])
            dma(nc.sync, 0.0050, wq_sb[:, 2:4], wq_d[:, 2:4])
            dma(nc.sync, 0.0052, wk_sb, wk_d[:, :, :])
            dma(nc.gpsimd, 0.0050, wv_sb, wv_d[:, :, :])
            dma(nc.gpsimd, 0.0054, wo_sb, wo_d[:, :, :])
            dma(nc.gpsimd, 0.0056, id_sb, id_d[:, :])
            # stage C (batch-1 x8 + batch-0 f32 x) is issued after
            # phase_qkv(0) below, behind a probe on batch-0's hn.

            # bn_stats floors: stage-A arrival estimates.
            arrive_ms = {
                0: {(ct, h): [0.0035, 0.0040, 0.0042, 0.0046][ct]
                    for ct in range(CT) for h in range(2)},
                1: {(ct, h): 0.0140 + 0.0010 * ct for ct in range(CT)
                    for h in range(2)},
            }

            eps_sb = consts.tile([GPT, 1], F32, tag="eps")
            nc.vector.memset(eps_sb, EPS)
            ebias_sb = consts.tile([128, 1], F32, tag="ebias")
            nc.vector.memset(ebias_sb, -EXP_BIAS)
            # ones for the DoubleRow row-sum; 16 columns because dual-fp8
            # LDWEIGHTS needs the pair-dim step to be a multiple of 16B.
            ones8_sb = consts.tile([128, 2, 16], F8, tag="ones8")
            nc.vector.memset(ones8_sb, 1.0)
            # Warm the ACT tables used later (first use of a function
            # pays the table load) while DMAs stream.
            for wf, wname in ((AF.Sqrt, "sqw"), (AF.Exp, "exw"),
                              (AF.Ln, "lnw"), (AF.Identity, "idw")):
                wt = consts.tile([GPT, 1], F32, tag=wname)
                nc.scalar.activation(out=wt, in_=eps_sb, func=wf,
                                     bias=0.0, scale=1.0)

            # Per-batch state carried across the phase interleave below.
            st = [dict() for _ in range(NB)]

            def phase_norm(b):
                """GroupNorm: per-tile bn_stats as DMAs land, ONE batched
                join (x8 tiles all land within ~2us of each other, so a
                single join has fewer DVE ops and fewer in-order stalls
                than per-tile chains)."""
                x8_sb = x8_sbs[b]
                hn_sb = big.tile([128, CT, N], BF16, tag="hn", bufs=2,
                                 name=f"hn{b}")
                gstats = small.tile([GPT, CT, 2], F32, tag="gstats",
                                    name=f"gstats{b}")
                for ct in range(CT):
                    stats = small.tile([128, 2, 6], F32, tag=f"bnst{ct}",
                                       name=f"bnst_{b}_{ct}")
                    ts = small.tile([128, 2], F32, tag=f"ts{ct}",
                                    name=f"ts_{b}_{ct}")
                    mv = small.tile([128, 2], F32, tag=f"mv{ct}",
                                    name=f"mv_{b}_{ct}")
                    for h in range(2):
                        with tc.tile_wait_until(arrive_ms[b][(ct, h)],
                                                enable=True):
                            nc.vector.bn_stats(
                                out=stats[:, h],
                                in_=x8_sb[:, ct, h * 512:(h + 1) * 512],
                            )
                    nc.vector.bn_aggr(out=mv, in_=stats)
                    nc.vector.tensor_copy(ts[:, 0:1], mv[:, 0:1])
                    nc.vector.tensor_mul(ts[:, 1:2], mv[:, 0:1], mv[:, 0:1])
                    nc.vector.tensor_add(ts[:, 1:2], ts[:, 1:2], mv[:, 1:2])
                    ps = psum.tile([GPT, 2], F32, tag="mm", bufs=6,
                                   name=f"stat_ps_{b}_{ct}")
                    nc.tensor.matmul(ps, lhsT=S_sb, rhs=ts,
                                     start=True, stop=True)
                    nc.vector.tensor_copy(gstats[:, ct], ps)
                # join: group mean / rstd for all tiles at once
                gm = small.tile([GPT, CT, 2], F32, tag="gm", name=f"gm{b}")
                nc.vector.tensor_scalar_mul(gm, gstats, 1.0 / GS)
                tmp8 = small.tile([GPT, CT], F32, tag="tmp8", name=f"tmp8{b}")
                nc.vector.tensor_mul(tmp8, gm[:, :, 0], gm[:, :, 0])
                nc.vector.tensor_sub(gm[:, :, 1], gm[:, :, 1], tmp8)
                nc.scalar.activation(out=gm[:, :, 1], in_=gm[:, :, 1],
                                     func=AF.Sqrt, bias=eps_sb, scale=1.0)
                nc.vector.reciprocal(gm[:, :, 1], gm[:, :, 1])
                AB = small.tile([128, CT, 2], F32, tag="AB", name=f"AB{b}")
                for ct in range(CT):
                    ps2 = psum.tile([128, 2], F32, tag="mm", bufs=6,
                                    name=f"ab_ps_{b}_{ct}")
                    nc.tensor.matmul(ps2, lhsT=ST_sb, rhs=gm[:, ct],
                                     start=True, stop=True)
                    nc.vector.tensor_copy(AB[:, ct], ps2)
                A_sb = small.tile([128, CT], F32, tag="A", name=f"A{b}")
                B_sb = small.tile([128, CT], F32, tag="B", name=f"B{b}")
                nc.vector.tensor_mul(A_sb, AB[:, :, 1], nsc_sb)
                nc.vector.tensor_mul(B_sb, AB[:, :, 0], A_sb)
                nc.vector.tensor_sub(B_sb, nbi_sb, B_sb)
                for ct in range(CT):
                    # hn = x*A + B (bf16 out); two tiles on GpSimd (slow
                    # but idle), two on DVE's fast path.
                    eng = nc.gpsimd if ct < 2 else nc.vector
                    eng.tensor_scalar(
                        out=hn_sb[:, ct], in0=x8_sb[:, ct],
                        scalar1=A_sb[:, ct:ct + 1],
                        scalar2=B_sb[:, ct:ct + 1],
                        op0=mybir.AluOpType.mult, op1=mybir.AluOpType.add,
                    )
                st[b]["hn"] = hn_sb

            def phase_qkv(b):
                """q, k (fp8 out) in [c, n]; vT (fp8 out) in [n, c]."""
                hn_sb = st[b]["hn"]
                q_sb = big.tile([128, CT, N], F8, tag="q", bufs=2,
                                name=f"q{b}")
                k_sb = big.tile([128, CT, N], F8, tag="k", bufs=2,
                                name=f"k{b}")
                evict_i = 0
                for wname, w_sb, dst, bias_sb, use_b in (
                        ("q", wq_sb, q_sb, bqt_sb, use_bq),
                        ("k", wk_sb, k_sb, bkt_sb, use_bk)):
                    for ot in range(CT):
                        pss = [psum.tile([128, 512], F32, tag="mm", bufs=6,
                                         name=f"{wname}_ps_{b}_{ot}_{ic}")
                               for ic in range(IC)]
                        for ct in range(CT):
                            # floor at this weight chunk's DMA arrival so
                            # the in-order PE stream is not scheduled
                            # ahead of data (wq ct0/1 land ~4.5 sched-us;
                            # wq ct2/3 and wk ~9.5).
                            wfl = (0.0 if b or wname != "q" else
                                   (0.0048 if ct < 2 else 0.0096))
                            if wname == "k" and b == 0:
                                wfl = 0.0098
                            with tc.tile_wait_until(wfl, enable=(b == 0)):
                                for ic in range(IC):
                                    nc.tensor.matmul(
                                        pss[ic],
                                        lhsT=w_sb[:, ct, ot * 128:(ot + 1) * 128],
                                        rhs=hn_sb[:, ct, ic * 512:(ic + 1) * 512],
                                        start=(ct == 0), stop=(ct == CT - 1),
                                    )
                        for ic in range(IC):
                            out = dst[:, ot, ic * 512:(ic + 1) * 512]
                            if use_b:
                                if evict_i % 2 == 0:
                                    nc.vector.tensor_scalar_add(
                                        out, pss[ic], bias_sb[:, ot:ot + 1])
                                else:
                                    nc.scalar.activation(
                                        out=out, in_=pss[ic], func=AF.Identity,
                                        bias=bias_sb[:, ot:ot + 1], scale=1.0)
                            else:
                                if evict_i % 2 == 0:
                                    nc.vector.tensor_copy(out, pss[ic])
                                else:
                                    nc.scalar.activation(
                                        out=out, in_=pss[ic], func=AF.Identity,
                                        bias=0.0, scale=1.0)
                            evict_i += 1
                vT_sb = big.tile([128, NT, 512], F8, tag="vT", bufs=2,
                                 name=f"vT{b}")
                for nt in range(NT):
                    ps = psum.tile([128, 512], F32, tag="mm", bufs=6,
                                   name=f"v_ps_{b}_{nt}")
                    with tc.tile_wait_until(0.0105, enable=(b == 0)):
                        for ct in range(CT):
                            nc.tensor.matmul(
                                ps,
                                lhsT=hn_sb[:, ct, nt * 128:(nt + 1) * 128],
                                rhs=wv_sb[:, ct, :],
                                start=(ct == 0), stop=(ct == CT - 1),
                            )
                    if nt % 2 == 0:
                        nc.vector.tensor_copy(vT_sb[:, nt], ps)
                    else:
                        nc.scalar.activation(out=vT_sb[:, nt], in_=ps,
                                             func=AF.Identity, bias=0.0,
                                             scale=1.0)
                st[b]["q"], st[b]["k"], st[b]["vT"] = q_sb, k_sb, vT_sb

            def phase_attn(b):
                """scores->exp (fp8), row sums, AV, all DoubleRow fp8."""
                q_sb, k_sb, vT_sb = st[b]["q"], st[b]["k"], st[b]["vT"]
                eTs = [big.tile([128, NT, 512], F8, tag="eT", bufs=4,
                                name=f"eT_{b}_{ic}") for ic in range(IC)]
                for jt in range(NT):
                    pss = [psum.tile([128, 512], F32, tag="mm", bufs=6,
                                     name=f"sc_ps_{b}_{jt}_{ic}")
                           for ic in range(IC)]
                    for cp in range(CP):
                        for ic in range(IC):
                            nc.tensor.matmul(
                                pss[ic],
                                lhsT=k_sb[:, 2 * cp:2 * cp + 2,
                                          jt * 128:(jt + 1) * 128],
                                rhs=q_sb[:, 2 * cp:2 * cp + 2,
                                         ic * 512:(ic + 1) * 512],
                                start=(cp == 0), stop=(cp == CP - 1),
                                perf_mode=DR,
                            )
                    for ic in range(IC):
                        nc.scalar.activation(
                            out=eTs[ic][:, jt], in_=pss[ic], func=AF.Exp,
                            scale=scale, bias=ebias_sb,
                        )
                # r[i] = sum_j eT[j, i] over the fp8 eT the AV GEMM sees
                rs_pss = [psum.tile([16, 512], F32, tag="small", bufs=2,
                                    name=f"rs_ps_{b}_{ic}") for ic in range(IC)]
                for jp in range(JP):
                    for ic in range(IC):
                        nc.tensor.matmul(
                            rs_pss[ic], lhsT=ones8_sb,
                            rhs=eTs[ic][:, 2 * jp:2 * jp + 2, :],
                            start=(jp == 0), stop=(jp == JP - 1),
                            perf_mode=DR,
                        )
                rsums, rinvs = [], []
                for ic in range(IC):
                    lr_sb = small.tile([1, 512], F32, tag="lnr", bufs=2,
                                       name=f"lnr_{b}_{ic}")
                    nc.scalar.activation(out=lr_sb, in_=rs_pss[ic][0:1, :],
                                         func=AF.Ln)
                    rinv_sb = small.tile([1, 512], F32R, tag="rinv", bufs=2,
                                         name=f"rinv_{b}_{ic}")
                    nc.scalar.activation(out=rinv_sb, in_=lr_sb, func=AF.Exp,
                                         scale=-1.0)
                    rinvs.append(rinv_sb)
                    if use_bv:
                        rsum_sb = small.tile([1, 512], F32R, tag="rsum",
                                             bufs=2, name=f"rsum_{b}_{ic}")
                        nc.vector.tensor_copy(rsum_sb, rs_pss[ic][0:1, :])
                        rsums.append(rsum_sb)

                avns = [big.tile([128, CT, 512], F8, tag="avn", bufs=4,
                                 name=f"avn_{b}_{ic}") for ic in range(IC)]
                av_pss = []
                bc_pss = []
                for ct in range(CT):
                    pss = [psum.tile([128, 512], F32, tag="mm", bufs=6,
                                     name=f"av_ps_{b}_{ct}_{ic}")
                           for ic in range(IC)]
                    av_pss.append(pss)
                    for jp in range(JP):
                        for ic in range(IC):
                            nc.tensor.matmul(
                                pss[ic],
                                lhsT=vT_sb[:, 2 * jp:2 * jp + 2,
                                           ct * 128:(ct + 1) * 128],
                                rhs=eTs[ic][:, 2 * jp:2 * jp + 2, :],
                                start=(jp == 0),
                                stop=(jp == JP - 1 and not use_bv),
                                perf_mode=DR,
                            )
                    if use_bv:
                        for ic in range(IC):
                            nc.tensor.matmul(
                                pss[ic],
                                lhsT=bvr_sb[0:1, ct * 128:(ct + 1) * 128],
                                rhs=rsums[ic], start=False, stop=True,
                                skip_group_check=True,
                            )
                    if ct == 0:
                        # broadcast 1/r across partitions; placed after
                        # the first AV group so the PE does not idle on
                        # the ACT ln/exp chain above.
                        for ic in range(IC):
                            bc_ps = psum.tile([128, 512], F32, tag="mm",
                                              bufs=6, name=f"bc_ps_{b}_{ic}")
                            nc.tensor.matmul(bc_ps, lhsT=ones1_sb,
                                             rhs=rinvs[ic],
                                             start=True, stop=True)
                            bc_pss.append(bc_ps)
                rinvbs = []
                for ic in range(IC):
                    rinvb_sb = small.tile([128, 512], F32, tag="rinvb", bufs=4,
                                          name=f"rinvb_{b}_{ic}")
                    nc.vector.tensor_copy(rinvb_sb, bc_pss[ic])
                    rinvbs.append(rinvb_sb)
                for ct in range(CT):
                    for ic in range(IC):
                        nc.vector.tensor_mul(avns[ic][:, ct], av_pss[ct][ic],
                                             rinvbs[ic])
                st[b]["avn"] = avns

            def phase_proj(b):
                """y = Wo av + x (+bo), residual via identity matmul."""
                x_sb = x_sbs[b]
                avns = st[b]["avn"]
                for ot in range(CT):
                    pss = [psum.tile([128, 512], F32, tag="mm", bufs=6,
                                     name=f"pr_ps_{b}_{ot}_{ic}")
                           for ic in range(IC)]
                    for cp in range(CP):
                        for ic in range(IC):
                            nc.tensor.matmul(
                                pss[ic],
                                lhsT=wo_sb[:, 2 * cp:2 * cp + 2,
                                           ot * 128:(ot + 1) * 128],
                                rhs=avns[ic][:, 2 * cp:2 * cp + 2, :],
                                start=(cp == 0), stop=False,
                                perf_mode=DR, skip_group_check=True,
                            )
                    for ic in range(IC):
                        nc.tensor.matmul(
                            pss[ic], lhsT=id_sb,
                            rhs=x_sb[:, ot, ic * 512:(ic + 1) * 512],
                            start=False, stop=(not use_bo),
                            skip_group_check=True,
                        )
                    if use_bo:
                        for ic in range(IC):
                            nc.tensor.matmul(
                                pss[ic],
                                lhsT=bor_sb[0:1, ot * 128:(ot + 1) * 128],
                                rhs=ones512_sb, start=False, stop=True,
                                skip_group_check=True,
                            )
                    y_sb = big.tile([128, N], F32, tag="y", bufs=4,
                                    name=f"y_{b}_{ot}")
                    for ic in range(IC):
                        out = y_sb[:, ic * 512:(ic + 1) * 512]
                        if ic % 2 == 0:
                            nc.scalar.activation(out=out, in_=pss[ic],
                                                 func=AF.Identity, bias=0.0,
                                                 scale=1.0)
                        else:
                            nc.vector.tensor_copy(out, pss[ic])
                    yqs = ([nc.sync, nc.gpsimd, nc.sync, nc.gpsimd] if b == 0
                           else [nc.sync, nc.gpsimd, nc.scalar, nc.sync])
                    yqs[ot].dma_start(out=y_d[b, :, ot, :], in_=y_sb)

            # ---- interleaved build: issue order is scheduler priority ----
            phase_norm(0)
            # stage C: batch-1 x8 + batch-0 f32 x (~3 MB), gated on the
            # second half of wq having landed (so C can't steal DMA
            # bandwidth from batch-0's x8).
            with tc.tile_wait_until(0.0072, enable=True):
                nc.sync.dma_start(out=probes[:, 8:9],
                                  in_=wq_sb[0:1, 3, 511:512])
                nc.gpsimd.dma_start(out=probes[:, 12:13],
                                    in_=wq_sb[0:1, 3, 510:511])
            for ct in range(CT):
                dma([nc.sync, nc.gpsimd][ct % 2], 0.0075 + 0.0003 * ct,
                    x8_sbs[1][:, ct], x8_d[1, :, ct])
            dma(nc.sync, 0.0082, x_sbs[0][:, 0:2], x_d[0, :, 0:2])
            dma(nc.gpsimd, 0.0082, x_sbs[0][:, 2:4], x_d[0, :, 2:4])
            phase_qkv(0)
            # batch-1 groupnorm issued EARLY so its small DVE chain
            # outranks batch-0's eviction stream and hides under batch-0's
            # attention (its x8 gates it at runtime anyway).
            phase_norm(1)
            phase_attn(0)
            # stage D: batch-1 f32 x on the scalar queue, behind the ACT
            # stream's batch-0 score exps (fires ~mid-attention).
            dma(nc.scalar, 0.0220, x_sbs[1][:, 0:2], x_d[1, :, 0:2])
            dma(nc.scalar, 0.0230, x_sbs[1][:, 2:4], x_d[1, :, 2:4])
            phase_proj(0)
            phase_qkv(1)
            phase_attn(1)
            phase_proj(1)
    return nc


_CACHE = {}


def _get_nc(use_bq=False, use_bk=False, use_bv=False, use_bo=False):
    key = (use_bq, use_bk, use_bv, use_bo)
    if key not in _CACHE:
        _CACHE[key] = build_nc(*key)
    return _CACHE[key]


def prepare(x, norm_scale, norm_bias, wq, bq, wk, bk, wv, bv, wo, bo):
    """Host-side prep: returns (in_maps, flags)."""
    x = np.ascontiguousarray(np.asarray(x, dtype=np.float32))
    f32 = lambda a: np.asarray(a, dtype=np.float32)
    norm_scale, norm_bias = f32(norm_scale), f32(norm_bias)
    wq, wk, wv, wo = f32(wq), f32(wk), f32(wv), f32(wo)
    bq, bk, bv, bo = f32(bq), f32(bk), f32(bv), f32(bo)

    # [C, C] w  ->  wT[c, o] arranged [p, ct, o]
    def arr_w(w, dt):
        a = np.ascontiguousarray(w.T.reshape(CT, 128, C).transpose(1, 0, 2))
        return np.ascontiguousarray(a.astype(dt))

    # [C] vec (channel-tile major) -> [p, ct]
    def arr_c(v):
        return np.ascontiguousarray(v.reshape(CT, 128).T)

    S = np.zeros((128, GPT), np.float32)
    S[np.arange(128), np.arange(128) // GS] = 1.0
    pk1 = np.concatenate(
        [S, arr_c(norm_scale), arr_c(norm_bias), arr_c(bq), arr_c(bk)], axis=1)
    pk2 = np.concatenate(
        [np.ones(128, np.float32), np.ones(512, np.float32),
         bo.reshape(C), bv.reshape(C)]).reshape(1, -1)
    common = {
        "wqT": arr_w(wq, ml_dtypes.bfloat16),
        "wkT": arr_w(wk, ml_dtypes.bfloat16),
        "wvT": arr_w(wv, ml_dtypes.bfloat16),
        "woT8": arr_w(wo, ml_dtypes.float8_e4m3),
        "idm": np.ascontiguousarray(np.eye(128, dtype=np.float32)),
        "pk1": np.ascontiguousarray(pk1),
        "pk2": np.ascontiguousarray(pk2),
        "ST": np.ascontiguousarray(S.T),
    }

    # x: (B, C, H, W) -> per core [NB, p, ct, n]
    xf = x.reshape(B, C, N).reshape(B, CT, 128, N).transpose(0, 2, 1, 3)
    x8f = np.ascontiguousarray(xf.astype(ml_dtypes.bfloat16))
    in_maps = [
        {**common,
         "x": np.ascontiguousarray(xf[i * NB:(i + 1) * NB]),
         "x8": np.ascontiguousarray(x8f[i * NB:(i + 1) * NB])}
        for i in range(NCORES)
    ]
    flags = (bool(np.any(bq != 0.0)), bool(np.any(bk != 0.0)),
             bool(np.any(bv != 0.0)), bool(np.any(bo != 0.0)))
    return in_maps, flags


def assemble(results):
    y = np.empty((B, C, N), np.float32)
    for i in range(NCORES):
        yc = results[i]["y"]  # [NB, 128, CT, N]
        y[i * NB:(i + 1) * NB] = (
            yc.transpose(0, 2, 1, 3).reshape(NB, C, N))
    return y.reshape(B, C, H, W)


def kernel(x, norm_scale, norm_bias, wq, bq, wk, bk, wv, bv, wo, bo):
    in_maps, flags = prepare(x, norm_scale, norm_bias, wq, bq,
                             wk, bk, wv, bv, wo, bo)
    nc = _get_nc(*flags)
    res = run_bass_kernel_spmd(nc, in_maps, list(range(NCORES)))
    return assemble(res.results)
